# revision 30
# baseline (speedup 1.0000x reference)
"""GAT layer kernel for Trainium2 (Bass/Tile), 8-core data-parallel over batch.

Reference (B=16, N=1024, IN_DIM=128, H=4, D=64):
    h = (x @ W).reshape(B,N,H,D)
    e_src/e_dst = einsum('bnhd,hd->bnh', h, a_src/a_dst)
    e[b,i,j,h] = leakyrelu(e_src[b,i,h] + e_dst[b,j,h], 0.2)
    alpha = softmax_j(where(adj[i,j], e, -inf));  out = alpha @ h

Kernel strategy (per core, 2 batches):
  Softmax shift-invariance: with y = s_i + d_j, lrelu(y) = 0.2 s_i + 0.2 d_j
  + 0.8 relu(y); the 0.2 s_i term is constant over j and cancels. So the
  (unnormalized) score reduces to
      PT[j,i] = max(u8_i * V_j, w_j) * m[j,i]
  with u8 = exp(0.8 e_src), V = exp(e_dst), w = exp(0.2 e_dst): one fused DVE
  tensor_scalar (mult, max) per (b,jc,h) against a partition-broadcast u8
  tile, plus one mask multiply shared across the 4 heads. A second path runs
  entirely on PE (rank-1 outer product u8 x V) + GPSIMD (fused (z max w)*m),
  soaking otherwise-idle engines. Row-sums ride separate 1-column matmuls;
  normalization is a batched reciprocal + broadcast multiply.
  All heavy matmuls use bf16 or fp32r (1 PE cycle/row vs 4 for fp32).
"""

import os
import sys
from contextlib import ExitStack

import numpy as np
import ml_dtypes

for _p in ("/opt/trn_rl_repo", "/root/.axon_site/_ro/trn_rl_repo"):
    if os.path.isdir(_p) and _p not in sys.path:
        sys.path.insert(0, _p)

import concourse.bass as bass
import concourse.mybir as mybir
import concourse.tile as tile

F32 = mybir.dt.float32
F32R = mybir.dt.float32r
BF16 = mybir.dt.bfloat16
AF = mybir.ActivationFunctionType
ALU = mybir.AluOpType
NPBF = ml_dtypes.bfloat16

B, N, IN_DIM, H, D = 16, 1024, 128, 4, 64
HD = H * D            # 256
NCORES = 8
BL = B // NCORES      # 2 batches per core
NTC = N // 128        # 8 chunks of 128

# (b, jc) score groups handled by the PE-outer-product + GPSIMD path;
# the rest go through the DVE tensor_scalar path.
POOL_GROUPS = set()
DEBUG_TAPS = False


def _split_excess_waits(nc, max_waits=1):
    """Walrus codegen rejects compute instructions carrying more than one
    sync wait. Move the extras onto engine-matched NoOps inserted
    immediately before the instruction."""
    def _steal_nop(engine):
        engine.nop()
        for fn in nc.m.functions:
            for blk in fn.blocks:
                il = blk.instructions
                if il and type(il[-1]).__name__ == "InstNoOp":
                    nop = il[-1]
                    blk.instructions = il[:-1]
                    return nop
        raise RuntimeError("could not locate appended nop")

    for fn in nc.m.functions:
        for blk in fn.blocks:
            il = list(blk.instructions)
            out = []
            changed = False
            for inst in il:
                si = inst.sync_info
                if (type(inst).__name__ != "InstNoOp" and si is not None
                        and len(si.on_wait) > max_waits):
                    waits = list(si.on_wait)
                    for w in waits[max_waits:]:
                        nop = _steal_nop(nc.engines[inst.engine])
                        nop.sync_info = mybir.SyncInfo(on_wait=[w], on_update=[])
                        out.append(nop)
                    inst.sync_info = mybir.SyncInfo(
                        on_wait=waits[:max_waits], on_update=list(si.on_update))
                    changed = True
                out.append(inst)
            if changed:
                blk.instructions = out


def build_gat_program():
    nc = bass.Bass("TRN2", target_bir_lowering=False, debug=False)
    xT_d = nc.dram_tensor("xT", (BL, IN_DIM, N), F32R, kind="ExternalInput").ap()
    W_d = nc.dram_tensor("W", (IN_DIM, HD), F32R, kind="ExternalInput").ap()
    WAcat_d = nc.dram_tensor("WAcat", (IN_DIM, 36), F32R, kind="ExternalInput").ap()
    maskT_d = nc.dram_tensor("maskT", (N, N), BF16, kind="ExternalInput").ap()
    onehot_d = nc.dram_tensor("onehot", (4, 4 * 128), F32R, kind="ExternalInput").ap()
    out_d = nc.dram_tensor("out", (BL, N, HD), BF16, kind="ExternalOutput").ap()
    taps = {}
    if DEBUG_TAPS:
        taps["u8bc"] = nc.dram_tensor("t_u8bc", (128, N), BF16, kind="ExternalOutput").ap()

        taps["vcol"] = nc.dram_tensor("t_vcol", (128, NTC, H), F32, kind="ExternalOutput").ap()
        taps["wcol"] = nc.dram_tensor("t_wcol", (128, NTC, H), F32, kind="ExternalOutput").ap()
        taps["qwm"] = nc.dram_tensor("t_qwm", (128, H, N), BF16, kind="ExternalOutput").ap()
        taps["rs"] = nc.dram_tensor("t_rs", (128, 32), F32, kind="ExternalOutput").ap()
        taps["haug"] = nc.dram_tensor("t_haug", (128, NTC, HD), BF16, kind="ExternalOutput").ap()

    with tile.TileContext(nc) as tc:
        with ExitStack() as ctx:
            _gat_body(ctx, tc, out_d, xT_d, W_d, WAcat_d, maskT_d, onehot_d,
                      taps)
    _split_excess_waits(nc)
    return nc


def _gat_body(ctx, tc, out_d, xT_d, W_d, WAcat_d, maskT_d, onehot_d, taps=None):
    nc = tc.nc

    consts = ctx.enter_context(tc.tile_pool(name="consts", bufs=1))
    persist = ctx.enter_context(tc.tile_pool(name="persist", bufs=1))
    qt_pool = ctx.enter_context(tc.tile_pool(name="qt", bufs=4))
    qwm_pool = ctx.enter_context(tc.tile_pool(name="qwm", bufs=6))
    osb_pool = ctx.enter_context(tc.tile_pool(name="osb", bufs=3))
    rcl_pool = ctx.enter_context(tc.tile_pool(name="rcl", bufs=3))
    ps_z = ctx.enter_context(tc.tile_pool(name="ps_z", bufs=2, space="PSUM"))
    ps_p1 = ctx.enter_context(tc.tile_pool(name="ps_p1", bufs=1, space="PSUM"))
    ps_acc = ctx.enter_context(tc.tile_pool(name="ps_acc", bufs=1, space="PSUM"))

    # ---- constants / inputs resident in SBUF ----
    # tiny weight tensors first so phase 1 isn't stuck behind bulk transfers
    WAcat_sb = consts.tile([128, 36], F32R)
    nc.sync.dma_start(out=WAcat_sb, in_=WAcat_d)
    onehot_sb = consts.tile([4, 4 * 128], F32R)
    nc.sync.dma_start(out=onehot_sb, in_=onehot_d)
    xT_sb = consts.tile([128, BL, N], F32R)
    for b in range(BL):
        nc.sync.dma_start(out=xT_sb[:, b, :], in_=xT_d[b])
    W_sb = consts.tile([128, HD], F32R)
    nc.sync.dma_start(out=W_sb, in_=W_d)
    ones_col = consts.tile([128, 1], BF16)
    nc.vector.memset(ones_col, 1.0)
    maskT_sb = consts.tile([128, NTC, N], BF16)
    nc.sync.dma_start(
        out=maskT_sb,
        in_=maskT_d.rearrange("(jc p) i -> p jc i", p=128))

    # ---- persistent per-batch intermediates ----
    haug_sb = persist.tile([128, BL, NTC, HD], BF16)   # [j-in-chunk, b, jc, h*64+d]
    srow_sb = persist.tile([4, BL, N], F32R)           # raw e_src rows
    Vcol_sb = persist.tile([128, BL, NTC, H], F32)     # exp(e_dst) cols
    wcol_sb = persist.tile([128, BL, NTC, H], F32)     # exp(0.2 e_dst) cols
    U8bc = persist.tile([128, BL, H, N], BF16)         # u8 broadcast over parts

    # ---- phase 1: E = x @ WAcat (rows + cols), haug = x @ W ----
    for b in range(BL):
        # E rows [a=src4+dst4, t] via two 512-col halves (z-pool slots)
        for half in range(2):
            e8 = ps_z.tile([128, 512], F32, tag="z")
            nc.tensor.matmul(e8[0:36, :], lhsT=WAcat_sb,
                             rhs=xT_sb[:, b, half * 512:(half + 1) * 512],
                             start=True, stop=True)
            sl = slice(half * 512, (half + 1) * 512)
            nc.scalar.activation(srow_sb[0:4, b, sl], e8[0:4, :], AF.Copy,
                                 bias=0.0, scale=1.0)
        # E cols [t, a] per 128-chunk; exp into V / w columns
        ecol_slot = ps_z.tile([128, 512], F32, tag="z", name=f"ecol_{b}")
        ecol = ecol_slot[:, 0:NTC * 36]
        for tcc in range(NTC):
            nc.tensor.matmul(ecol[:, tcc * 36:(tcc + 1) * 36],
                             lhsT=xT_sb[:, b, tcc * 128:(tcc + 1) * 128],
                             rhs=WAcat_sb, start=True, stop=True)
        dstv = ecol.rearrange("p (t a) -> p t a", t=NTC)[:, :, 32:36]
        nc.scalar.activation(Vcol_sb[:, b], dstv, AF.Exp, bias=0.0, scale=1.0)
        nc.scalar.activation(wcol_sb[:, b], dstv, AF.Exp, bias=0.0, scale=0.2)
        # broadcast e_src row h across partitions via one-hot K=4 matmul,
        # then exp(0.8 x) straight from PSUM into the bf16 U8bc tile
        for h in range(H):
            for half in range(2):
                sl = slice(half * 512, (half + 1) * 512)
                ebc = ps_z.tile([128, 512], F32, tag="z", name=f"ebc_{b}_{h}_{half}")
                nc.tensor.matmul(ebc, lhsT=onehot_sb[:, h * 128:(h + 1) * 128],
                                 rhs=srow_sb[0:4, b, sl], start=True, stop=True)
                nc.scalar.activation(U8bc[:, b, h, sl], ebc, AF.Exp,
                                     bias=0.0, scale=0.8)
        # haug[t, h*64+d] = h in bf16 for the alpha@h contraction
        for tcc in range(NTC):
            hp = ps_p1.tile([128, HD], F32, tag="haug")
            nc.tensor.matmul(hp, lhsT=xT_sb[:, b, tcc * 128:(tcc + 1) * 128],
                             rhs=W_sb, start=True, stop=True)
            nc.scalar.activation(haug_sb[:, b, tcc, :], hp, AF.Copy,
                                 bias=0.0, scale=1.0)

    # ---- phase 2: scores + alpha @ h ----
    for b in range(BL):
        # 4 oacc banks hold the 32 (ic,h) 64-col chains; rs holds row-sums
        obank = [ps_acc.tile([128, 512], F32, tag=f"oacc{k}", name=f"oacc{k}_{b}")
                 for k in range(4)]
        rs = ps_acc.tile([128, 32], F32, tag="rs")
        for jc in range(NTC):
            qwm = qwm_pool.tile([128, H, N], BF16, tag="qwm")
            if (b, jc) in POOL_GROUPS:
                qt = qt_pool.tile([128, H, N], BF16, tag="qt")
                for h in range(H):
                    # u8_i * V_j via ACT copy with column scale
                    nc.scalar.activation(qt[:, h, :], U8bc[:, b, h, :], AF.Copy,
                                         bias=0.0,
                                         scale=Vcol_sb[:, b, jc, h:h + 1])
                    # (u8*V max w) * m on GPSIMD, all SBUF
                    nc.gpsimd.scalar_tensor_tensor(
                        out=qwm[:, h, :], in0=qt[:, h, :],
                        scalar=wcol_sb[:, b, jc, h:h + 1],
                        in1=maskT_sb[:, jc, :],
                        op0=ALU.max, op1=ALU.mult)
            else:
                qt = qt_pool.tile([128, H, N], BF16, tag="qt")
                for h in range(H):
                    nc.vector.tensor_scalar(
                        out=qt[:, h, :], in0=U8bc[:, b, h, :],
                        scalar1=Vcol_sb[:, b, jc, h:h + 1],
                        scalar2=wcol_sb[:, b, jc, h:h + 1],
                        op0=ALU.mult, op1=ALU.max)
                nc.vector.tensor_tensor(
                    out=qwm, in0=qt,
                    in1=maskT_sb[:, jc, :].unsqueeze(1).broadcast_to((128, H, N)),
                    op=ALU.mult)
            # start=True zeroes a whole 2KB psum bank: only the first chain in
            # each bank starts the group, only the last one stops it.
            for h in range(H):
                for ic in range(NTC):
                    c = ic * 4 + h
                    lhsT = qwm[:, h, ic * 128:(ic + 1) * 128]
                    nc.tensor.matmul(
                        obank[c // 8][:, (c % 8) * 64:(c % 8 + 1) * 64],
                        lhsT=lhsT, rhs=haug_sb[:, b, jc, h * 64:(h + 1) * 64],
                        start=(jc == 0 and c % 8 == 0),
                        stop=(jc == NTC - 1 and c % 8 == 7))
                    nc.tensor.matmul(rs[:, c:c + 1], lhsT=lhsT, rhs=ones_col,
                                     start=(jc == 0 and c == 0),
                                     stop=(jc == NTC - 1 and c == 31))
        if taps and b == 0:
            nc.sync.dma_start(out=taps["u8bc"], in_=U8bc[:, 0, 0, :])

            nc.sync.dma_start(out=taps["vcol"], in_=Vcol_sb[:, 0])
            nc.sync.dma_start(out=taps["wcol"], in_=wcol_sb[:, 0])
            rs_tap = osb_pool.tile([128, 32], F32, tag="rstap")
            nc.vector.tensor_copy(rs_tap, rs)
            nc.sync.dma_start(out=taps["rs"], in_=rs_tap)
            nc.sync.dma_start(out=taps["haug"], in_=haug_sb[:, 0])
        for ic in range(NTC):
            rcl = rcl_pool.tile([128, 4], F32, tag="rcl")
            nc.vector.reciprocal(rcl, rs[:, ic * 4:(ic + 1) * 4])
            osb = osb_pool.tile([128, H, D], BF16, tag="osb")
            oslice = obank[ic // 2][:, (ic % 2) * 256:(ic % 2 + 1) * 256]
            if b == 0 or ic < 4:  # ACT norm except the very tail (idle DVE)
                for h in range(H):
                    nc.scalar.activation(
                        osb[:, h, :], oslice[:, h * 64:(h + 1) * 64], AF.Copy,
                        bias=0.0, scale=rcl[:, h:h + 1])
            else:
                nc.vector.tensor_tensor(
                    out=osb, in0=oslice.rearrange("p (h d) -> p h d", h=H),
                    in1=rcl.unsqueeze(2).broadcast_to((128, 4, D)), op=ALU.mult)
            nc.sync.dma_start(out=out_d[b, ic * 128:(ic + 1) * 128, :],
                              in_=osb.rearrange("p h d -> p (h d)"))


def prep_inputs(x, adj, W, a_src, a_dst):
    """Host-side prep: shard x over cores, build combined weight layouts."""
    x = np.asarray(x, np.float32)
    adj = np.asarray(adj)
    W = np.asarray(W, np.float32)
    a_src = np.asarray(a_src, np.float32)
    a_dst = np.asarray(a_dst, np.float32)

    maskT = np.ascontiguousarray(adj.T).astype(NPBF)
    Acat = np.zeros((HD, 36), np.float32)
    for h in range(H):
        Acat[h * D:(h + 1) * D, h] = a_src[h]
        Acat[h * D:(h + 1) * D, 32 + h] = a_dst[h]
    WAcat = np.ascontiguousarray(W @ Acat)  # (IN_DIM, 36): src at 0-3, dst at 32-35

    onehot = np.zeros((4, 4 * 128), np.float32)
    for h in range(H):
        onehot[h, h * 128:(h + 1) * 128] = 1.0

    in_maps = []
    for c in range(NCORES):
        xT = np.ascontiguousarray(x[c * BL:(c + 1) * BL].transpose(0, 2, 1))
        in_maps.append({"xT": xT, "W": W, "WAcat": WAcat, "maskT": maskT,
                        "onehot": onehot})
    return in_maps


_PROGRAM_CACHE = {}


def _get_program():
    if "nc" not in _PROGRAM_CACHE:
        _PROGRAM_CACHE["nc"] = build_gat_program()
    return _PROGRAM_CACHE["nc"]


def run_on_hw(inputs, trace=False):
    from concourse.bass_utils import run_bass_kernel_spmd
    nc = _get_program()
    in_maps = prep_inputs(**inputs)
    res = run_bass_kernel_spmd(nc, in_maps, list(range(NCORES)), trace=trace)
    out = np.concatenate(
        [np.asarray(res.results[c]["out"]).astype(np.float32)
         for c in range(NCORES)], axis=0)
    return out, res


def kernel(**inputs) -> np.ndarray:
    out, _ = run_on_hw(inputs, trace=False)
    return out


# revision 31
# speedup vs baseline: 1.0340x; 1.0340x over previous
"""GAT layer kernel for Trainium2 (Bass/Tile), 8-core data-parallel over batch.

Reference (B=16, N=1024, IN_DIM=128, H=4, D=64):
    h = (x @ W).reshape(B,N,H,D)
    e_src/e_dst = einsum('bnhd,hd->bnh', h, a_src/a_dst)
    e[b,i,j,h] = leakyrelu(e_src[b,i,h] + e_dst[b,j,h], 0.2)
    alpha = softmax_j(where(adj[i,j], e, -inf));  out = alpha @ h

Kernel strategy (per core, 2 batches):
  Softmax shift-invariance: with y = s_i + d_j, lrelu(y) = 0.2 s_i + 0.2 d_j
  + 0.8 relu(y); the 0.2 s_i term is constant over j and cancels. So the
  (unnormalized) score reduces to
      PT[j,i] = max(u8_i * V_j, w_j) * m[j,i]
  with u8 = exp(0.8 e_src), V = exp(e_dst), w = exp(0.2 e_dst): one fused DVE
  tensor_scalar (mult, max) per (b,jc,h) against a partition-broadcast u8
  tile, plus one mask multiply shared across the 4 heads. A second path runs
  entirely on PE (rank-1 outer product u8 x V) + GPSIMD (fused (z max w)*m),
  soaking otherwise-idle engines. Row-sums ride separate 1-column matmuls;
  normalization is a batched reciprocal + broadcast multiply.
  All heavy matmuls use bf16 or fp32r (1 PE cycle/row vs 4 for fp32).
"""

import os
import sys
from contextlib import ExitStack

import numpy as np
import ml_dtypes

for _p in ("/opt/trn_rl_repo", "/root/.axon_site/_ro/trn_rl_repo"):
    if os.path.isdir(_p) and _p not in sys.path:
        sys.path.insert(0, _p)

import concourse.bass as bass
import concourse.mybir as mybir
import concourse.tile as tile

F32 = mybir.dt.float32
F32R = mybir.dt.float32r
BF16 = mybir.dt.bfloat16
AF = mybir.ActivationFunctionType
ALU = mybir.AluOpType
NPBF = ml_dtypes.bfloat16

B, N, IN_DIM, H, D = 16, 1024, 128, 4, 64
HD = H * D            # 256
NCORES = 8
BL = B // NCORES      # 2 batches per core
NTC = N // 128        # 8 chunks of 128

# (b, jc) score groups handled by the PE-outer-product + GPSIMD path;
# the rest go through the DVE tensor_scalar path.
POOL_GROUPS = set()
DEBUG_TAPS = False


def _split_excess_waits(nc, max_waits=1):
    """Walrus codegen rejects compute instructions carrying more than one
    sync wait. Move the extras onto engine-matched NoOps inserted
    immediately before the instruction."""
    def _steal_nop(engine):
        engine.nop()
        for fn in nc.m.functions:
            for blk in fn.blocks:
                il = blk.instructions
                if il and type(il[-1]).__name__ == "InstNoOp":
                    nop = il[-1]
                    blk.instructions = il[:-1]
                    return nop
        raise RuntimeError("could not locate appended nop")

    for fn in nc.m.functions:
        for blk in fn.blocks:
            il = list(blk.instructions)
            out = []
            changed = False
            for inst in il:
                si = inst.sync_info
                if (type(inst).__name__ != "InstNoOp" and si is not None
                        and len(si.on_wait) > max_waits):
                    waits = list(si.on_wait)
                    for w in waits[max_waits:]:
                        nop = _steal_nop(nc.engines[inst.engine])
                        nop.sync_info = mybir.SyncInfo(on_wait=[w], on_update=[])
                        out.append(nop)
                    inst.sync_info = mybir.SyncInfo(
                        on_wait=waits[:max_waits], on_update=list(si.on_update))
                    changed = True
                out.append(inst)
            if changed:
                blk.instructions = out


def build_gat_program():
    nc = bass.Bass("TRN2", target_bir_lowering=False, debug=False)
    xT_d = nc.dram_tensor("xT", (BL, IN_DIM, N), F32R, kind="ExternalInput").ap()
    W_d = nc.dram_tensor("W", (IN_DIM, HD), F32R, kind="ExternalInput").ap()
    WAcat_d = nc.dram_tensor("WAcat", (IN_DIM, 36), F32R, kind="ExternalInput").ap()
    maskT_d = nc.dram_tensor("maskT", (N, N), BF16, kind="ExternalInput").ap()
    onehot_d = nc.dram_tensor("onehot", (4, 4 * 128), F32R, kind="ExternalInput").ap()
    out_d = nc.dram_tensor("out", (BL, N, HD), BF16, kind="ExternalOutput").ap()
    taps = {}
    if DEBUG_TAPS:
        taps["u8bc"] = nc.dram_tensor("t_u8bc", (128, N), BF16, kind="ExternalOutput").ap()

        taps["vcol"] = nc.dram_tensor("t_vcol", (128, NTC, H), F32, kind="ExternalOutput").ap()
        taps["wcol"] = nc.dram_tensor("t_wcol", (128, NTC, H), F32, kind="ExternalOutput").ap()
        taps["qwm"] = nc.dram_tensor("t_qwm", (128, H, N), BF16, kind="ExternalOutput").ap()
        taps["rs"] = nc.dram_tensor("t_rs", (128, 32), F32, kind="ExternalOutput").ap()
        taps["haug"] = nc.dram_tensor("t_haug", (128, NTC, HD), BF16, kind="ExternalOutput").ap()

    with tile.TileContext(nc) as tc:
        with ExitStack() as ctx:
            _gat_body(ctx, tc, out_d, xT_d, W_d, WAcat_d, maskT_d, onehot_d,
                      taps)
    _split_excess_waits(nc)
    return nc


def _gat_body(ctx, tc, out_d, xT_d, W_d, WAcat_d, maskT_d, onehot_d, taps=None):
    nc = tc.nc

    consts = ctx.enter_context(tc.tile_pool(name="consts", bufs=1))
    persist = ctx.enter_context(tc.tile_pool(name="persist", bufs=1))
    qt_pool = ctx.enter_context(tc.tile_pool(name="qt", bufs=4))
    qwm_pool = ctx.enter_context(tc.tile_pool(name="qwm", bufs=6))
    osb_pool = ctx.enter_context(tc.tile_pool(name="osb", bufs=3))
    rcl_pool = ctx.enter_context(tc.tile_pool(name="rcl", bufs=3))
    ps_z = ctx.enter_context(tc.tile_pool(name="ps_z", bufs=2, space="PSUM"))
    ps_p1 = ctx.enter_context(tc.tile_pool(name="ps_p1", bufs=1, space="PSUM"))
    ps_acc = ctx.enter_context(tc.tile_pool(name="ps_acc", bufs=1, space="PSUM"))

    # ---- constants / inputs resident in SBUF ----
    # tiny weight tensors first so phase 1 isn't stuck behind bulk transfers
    WAcat_sb = consts.tile([128, 36], F32R)
    nc.sync.dma_start(out=WAcat_sb, in_=WAcat_d)
    onehot_sb = consts.tile([4, 4 * 128], F32R)
    nc.sync.dma_start(out=onehot_sb, in_=onehot_d)
    xT_sb = consts.tile([128, BL, N], F32R)
    for b in range(BL):
        nc.sync.dma_start(out=xT_sb[:, b, :], in_=xT_d[b])
    W_sb = consts.tile([128, HD], F32R)
    nc.sync.dma_start(out=W_sb, in_=W_d)
    ones_col = consts.tile([128, 1], BF16)
    nc.vector.memset(ones_col, 1.0)
    maskT_sb = consts.tile([128, NTC, N], BF16)
    nc.sync.dma_start(
        out=maskT_sb,
        in_=maskT_d.rearrange("(jc p) i -> p jc i", p=128))

    # ---- persistent per-batch intermediates ----
    haug_sb = persist.tile([128, BL, NTC, HD], BF16)   # [j-in-chunk, b, jc, h*64+d]
    srow_sb = persist.tile([4, BL, N], F32R)           # raw e_src rows
    Vcol_sb = persist.tile([128, BL, NTC, H], F32)     # exp(e_dst) cols
    wcol_sb = persist.tile([128, BL, NTC, H], F32)     # exp(0.2 e_dst) cols
    U8bc = persist.tile([128, BL, H, N], BF16)         # u8 broadcast over parts

    # ---- phase 1: E = x @ WAcat (rows + cols), haug = x @ W ----
    for b in range(BL):
        # E rows [a=src4+dst4, t] via two 512-col halves (z-pool slots)
        for half in range(2):
            e8 = ps_z.tile([128, 512], F32, tag="z")
            nc.tensor.matmul(e8[0:36, :], lhsT=WAcat_sb,
                             rhs=xT_sb[:, b, half * 512:(half + 1) * 512],
                             start=True, stop=True)
            sl = slice(half * 512, (half + 1) * 512)
            nc.scalar.activation(srow_sb[0:4, b, sl], e8[0:4, :], AF.Copy,
                                 bias=0.0, scale=1.0)
        # E cols [t, a] per 128-chunk; exp into V / w columns
        ecol_slot = ps_z.tile([128, 512], F32, tag="z", name=f"ecol_{b}")
        ecol = ecol_slot[:, 0:NTC * 36]
        for tcc in range(NTC):
            nc.tensor.matmul(ecol[:, tcc * 36:(tcc + 1) * 36],
                             lhsT=xT_sb[:, b, tcc * 128:(tcc + 1) * 128],
                             rhs=WAcat_sb, start=True, stop=True)
        dstv = ecol.rearrange("p (t a) -> p t a", t=NTC)[:, :, 32:36]
        nc.scalar.activation(Vcol_sb[:, b], dstv, AF.Exp, bias=0.0, scale=1.0)
        nc.scalar.activation(wcol_sb[:, b], dstv, AF.Exp, bias=0.0, scale=0.2)
        # broadcast e_src row h across partitions via one-hot K=4 matmul,
        # then exp(0.8 x) straight from PSUM into the bf16 U8bc tile
        for h in range(H):
            for half in range(2):
                sl = slice(half * 512, (half + 1) * 512)
                ebc = ps_z.tile([128, 512], F32, tag="z", name=f"ebc_{b}_{h}_{half}")
                nc.tensor.matmul(ebc, lhsT=onehot_sb[:, h * 128:(h + 1) * 128],
                                 rhs=srow_sb[0:4, b, sl], start=True, stop=True)
                nc.scalar.activation(U8bc[:, b, h, sl], ebc, AF.Exp,
                                     bias=0.0, scale=0.8)
        # haug[t, h*64+d] = h in bf16 for the alpha@h contraction
        for tcc in range(NTC):
            hp = ps_p1.tile([128, HD], F32, tag="haug")
            nc.tensor.matmul(hp, lhsT=xT_sb[:, b, tcc * 128:(tcc + 1) * 128],
                             rhs=W_sb, start=True, stop=True)
            nc.scalar.activation(haug_sb[:, b, tcc, :], hp, AF.Copy,
                                 bias=0.0, scale=1.0)

    # ---- phase 2: scores + alpha @ h ----
    for b in range(BL):
        # 4 oacc banks hold the 32 (ic,h) 64-col chains; rs holds row-sums
        obank = [ps_acc.tile([128, 512], F32, tag=f"oacc{k}", name=f"oacc{k}_{b}")
                 for k in range(4)]
        rs = ps_acc.tile([128, 32], F32, tag="rs")
        for jc in range(NTC):
            qwm = qwm_pool.tile([128, H, N], BF16, tag="qwm")
            if (b, jc) in POOL_GROUPS:
                qt = qt_pool.tile([128, H, N], BF16, tag="qt")
                for h in range(H):
                    # u8_i * V_j via ACT copy with column scale
                    nc.scalar.activation(qt[:, h, :], U8bc[:, b, h, :], AF.Copy,
                                         bias=0.0,
                                         scale=Vcol_sb[:, b, jc, h:h + 1])
                    # (u8*V max w) * m on GPSIMD, all SBUF
                    nc.gpsimd.scalar_tensor_tensor(
                        out=qwm[:, h, :], in0=qt[:, h, :],
                        scalar=wcol_sb[:, b, jc, h:h + 1],
                        in1=maskT_sb[:, jc, :],
                        op0=ALU.max, op1=ALU.mult)
            else:
                qt = qt_pool.tile([128, H, N], BF16, tag="qt")
                for h in range(H):
                    nc.vector.tensor_scalar(
                        out=qt[:, h, :], in0=U8bc[:, b, h, :],
                        scalar1=Vcol_sb[:, b, jc, h:h + 1],
                        scalar2=wcol_sb[:, b, jc, h:h + 1],
                        op0=ALU.mult, op1=ALU.max)
                nc.vector.tensor_tensor(
                    out=qwm, in0=qt,
                    in1=maskT_sb[:, jc, :].unsqueeze(1).broadcast_to((128, H, N)),
                    op=ALU.mult)
            # start=True zeroes a whole 2KB psum bank: only the first chain in
            # each bank starts the group, only the last one stops it.
            for h in range(H):
                for ic in range(NTC):
                    c = ic * 4 + h
                    lhsT = qwm[:, h, ic * 128:(ic + 1) * 128]
                    nc.tensor.matmul(
                        obank[c // 8][:, (c % 8) * 64:(c % 8 + 1) * 64],
                        lhsT=lhsT, rhs=haug_sb[:, b, jc, h * 64:(h + 1) * 64],
                        start=(jc == 0 and c % 8 == 0),
                        stop=(jc == NTC - 1 and c % 8 == 7))
                    nc.tensor.matmul(rs[:, c:c + 1], lhsT=lhsT, rhs=ones_col,
                                     start=(jc == 0 and c == 0),
                                     stop=(jc == NTC - 1 and c == 31))
        if taps and b == 0:
            nc.sync.dma_start(out=taps["u8bc"], in_=U8bc[:, 0, 0, :])

            nc.sync.dma_start(out=taps["vcol"], in_=Vcol_sb[:, 0])
            nc.sync.dma_start(out=taps["wcol"], in_=wcol_sb[:, 0])
            rs_tap = osb_pool.tile([128, 32], F32, tag="rstap")
            nc.vector.tensor_copy(rs_tap, rs)
            nc.sync.dma_start(out=taps["rs"], in_=rs_tap)
            nc.sync.dma_start(out=taps["haug"], in_=haug_sb[:, 0])
        for ic in range(NTC):
            rcl = rcl_pool.tile([128, 4], F32, tag="rcl")
            nc.vector.reciprocal(rcl, rs[:, ic * 4:(ic + 1) * 4])
            osb = osb_pool.tile([128, H, D], BF16, tag="osb")
            oslice = obank[ic // 2][:, (ic % 2) * 256:(ic % 2 + 1) * 256]
            if b == 0:  # b0 norm on ACT (overlaps b1 scores); b1 on idle-tail DVE
                for h in range(H):
                    nc.scalar.activation(
                        osb[:, h, :], oslice[:, h * 64:(h + 1) * 64], AF.Copy,
                        bias=0.0, scale=rcl[:, h:h + 1])
            else:
                nc.vector.tensor_tensor(
                    out=osb, in0=oslice.rearrange("p (h d) -> p h d", h=H),
                    in1=rcl.unsqueeze(2).broadcast_to((128, 4, D)), op=ALU.mult)
            nc.sync.dma_start(out=out_d[b, ic * 128:(ic + 1) * 128, :],
                              in_=osb.rearrange("p h d -> p (h d)"))


def prep_inputs(x, adj, W, a_src, a_dst):
    """Host-side prep: shard x over cores, build combined weight layouts."""
    x = np.asarray(x, np.float32)
    adj = np.asarray(adj)
    W = np.asarray(W, np.float32)
    a_src = np.asarray(a_src, np.float32)
    a_dst = np.asarray(a_dst, np.float32)

    maskT = np.ascontiguousarray(adj.T).astype(NPBF)
    Acat = np.zeros((HD, 36), np.float32)
    for h in range(H):
        Acat[h * D:(h + 1) * D, h] = a_src[h]
        Acat[h * D:(h + 1) * D, 32 + h] = a_dst[h]
    WAcat = np.ascontiguousarray(W @ Acat)  # (IN_DIM, 36): src at 0-3, dst at 32-35

    onehot = np.zeros((4, 4 * 128), np.float32)
    for h in range(H):
        onehot[h, h * 128:(h + 1) * 128] = 1.0

    in_maps = []
    for c in range(NCORES):
        xT = np.ascontiguousarray(x[c * BL:(c + 1) * BL].transpose(0, 2, 1))
        in_maps.append({"xT": xT, "W": W, "WAcat": WAcat, "maskT": maskT,
                        "onehot": onehot})
    return in_maps


_PROGRAM_CACHE = {}


def _get_program():
    if "nc" not in _PROGRAM_CACHE:
        _PROGRAM_CACHE["nc"] = build_gat_program()
    return _PROGRAM_CACHE["nc"]


def run_on_hw(inputs, trace=False):
    from concourse.bass_utils import run_bass_kernel_spmd
    nc = _get_program()
    in_maps = prep_inputs(**inputs)
    res = run_bass_kernel_spmd(nc, in_maps, list(range(NCORES)), trace=trace)
    out = np.concatenate(
        [np.asarray(res.results[c]["out"]).astype(np.float32)
         for c in range(NCORES)], axis=0)
    return out, res


def kernel(**inputs) -> np.ndarray:
    out, _ = run_on_hw(inputs, trace=False)
    return out


# revision 32
# speedup vs baseline: 1.0374x; 1.0033x over previous
"""GAT layer kernel for Trainium2 (Bass/Tile), 8-core data-parallel over batch.

Reference (B=16, N=1024, IN_DIM=128, H=4, D=64):
    h = (x @ W).reshape(B,N,H,D)
    e_src/e_dst = einsum('bnhd,hd->bnh', h, a_src/a_dst)
    e[b,i,j,h] = leakyrelu(e_src[b,i,h] + e_dst[b,j,h], 0.2)
    alpha = softmax_j(where(adj[i,j], e, -inf));  out = alpha @ h

Kernel strategy (per core, 2 batches):
  Softmax shift-invariance: with y = s_i + d_j, lrelu(y) = 0.2 s_i + 0.2 d_j
  + 0.8 relu(y); the 0.2 s_i term is constant over j and cancels. So the
  (unnormalized) score reduces to
      PT[j,i] = max(u8_i * V_j, w_j) * m[j,i]
  with u8 = exp(0.8 e_src), V = exp(e_dst), w = exp(0.2 e_dst): one fused DVE
  tensor_scalar (mult, max) per (b,jc,h) against a partition-broadcast u8
  tile, plus one mask multiply shared across the 4 heads. A second path runs
  entirely on PE (rank-1 outer product u8 x V) + GPSIMD (fused (z max w)*m),
  soaking otherwise-idle engines. Row-sums ride separate 1-column matmuls;
  normalization is a batched reciprocal + broadcast multiply.
  All heavy matmuls use bf16 or fp32r (1 PE cycle/row vs 4 for fp32).
"""

import os
import sys
from contextlib import ExitStack

import numpy as np
import ml_dtypes

for _p in ("/opt/trn_rl_repo", "/root/.axon_site/_ro/trn_rl_repo"):
    if os.path.isdir(_p) and _p not in sys.path:
        sys.path.insert(0, _p)

import concourse.bass as bass
import concourse.mybir as mybir
import concourse.tile as tile

F32 = mybir.dt.float32
F32R = mybir.dt.float32r
BF16 = mybir.dt.bfloat16
AF = mybir.ActivationFunctionType
ALU = mybir.AluOpType
NPBF = ml_dtypes.bfloat16

B, N, IN_DIM, H, D = 16, 1024, 128, 4, 64
HD = H * D            # 256
NCORES = 8
BL = B // NCORES      # 2 batches per core
NTC = N // 128        # 8 chunks of 128

# (b, jc) score groups handled by the PE-outer-product + GPSIMD path;
# the rest go through the DVE tensor_scalar path.
POOL_GROUPS = set()
DEBUG_TAPS = False


def _split_excess_waits(nc, max_waits=1):
    """Walrus codegen rejects compute instructions carrying more than one
    sync wait. Move the extras onto engine-matched NoOps inserted
    immediately before the instruction."""
    def _steal_nop(engine):
        engine.nop()
        for fn in nc.m.functions:
            for blk in fn.blocks:
                il = blk.instructions
                if il and type(il[-1]).__name__ == "InstNoOp":
                    nop = il[-1]
                    blk.instructions = il[:-1]
                    return nop
        raise RuntimeError("could not locate appended nop")

    for fn in nc.m.functions:
        for blk in fn.blocks:
            il = list(blk.instructions)
            out = []
            changed = False
            for inst in il:
                si = inst.sync_info
                if (type(inst).__name__ != "InstNoOp" and si is not None
                        and len(si.on_wait) > max_waits):
                    waits = list(si.on_wait)
                    for w in waits[max_waits:]:
                        nop = _steal_nop(nc.engines[inst.engine])
                        nop.sync_info = mybir.SyncInfo(on_wait=[w], on_update=[])
                        out.append(nop)
                    inst.sync_info = mybir.SyncInfo(
                        on_wait=waits[:max_waits], on_update=list(si.on_update))
                    changed = True
                out.append(inst)
            if changed:
                blk.instructions = out


def build_gat_program():
    nc = bass.Bass("TRN2", target_bir_lowering=False, debug=False)
    xT_d = nc.dram_tensor("xT", (BL, IN_DIM, N), F32R, kind="ExternalInput").ap()
    W_d = nc.dram_tensor("W", (IN_DIM, HD), F32R, kind="ExternalInput").ap()
    WAcat_d = nc.dram_tensor("WAcat", (IN_DIM, 36), F32R, kind="ExternalInput").ap()
    maskT_d = nc.dram_tensor("maskT", (N, N), BF16, kind="ExternalInput").ap()
    onehot_d = nc.dram_tensor("onehot", (4, 4 * 128), F32R, kind="ExternalInput").ap()
    out_d = nc.dram_tensor("out", (BL, N, HD), BF16, kind="ExternalOutput").ap()
    taps = {}
    if DEBUG_TAPS:
        taps["u8bc"] = nc.dram_tensor("t_u8bc", (128, N), BF16, kind="ExternalOutput").ap()

        taps["vcol"] = nc.dram_tensor("t_vcol", (128, NTC, H), F32, kind="ExternalOutput").ap()
        taps["wcol"] = nc.dram_tensor("t_wcol", (128, NTC, H), F32, kind="ExternalOutput").ap()
        taps["qwm"] = nc.dram_tensor("t_qwm", (128, H, N), BF16, kind="ExternalOutput").ap()
        taps["rs"] = nc.dram_tensor("t_rs", (128, 32), F32, kind="ExternalOutput").ap()
        taps["haug"] = nc.dram_tensor("t_haug", (128, NTC, HD), BF16, kind="ExternalOutput").ap()

    with tile.TileContext(nc) as tc:
        with ExitStack() as ctx:
            _gat_body(ctx, tc, out_d, xT_d, W_d, WAcat_d, maskT_d, onehot_d,
                      taps)
    _split_excess_waits(nc)
    return nc


def _gat_body(ctx, tc, out_d, xT_d, W_d, WAcat_d, maskT_d, onehot_d, taps=None):
    nc = tc.nc

    consts = ctx.enter_context(tc.tile_pool(name="consts", bufs=1))
    persist = ctx.enter_context(tc.tile_pool(name="persist", bufs=1))
    qt_pool = ctx.enter_context(tc.tile_pool(name="qt", bufs=4))
    qwm_pool = ctx.enter_context(tc.tile_pool(name="qwm", bufs=6))
    osb_pool = ctx.enter_context(tc.tile_pool(name="osb", bufs=3))
    rcl_pool = ctx.enter_context(tc.tile_pool(name="rcl", bufs=3))
    ps_z = ctx.enter_context(tc.tile_pool(name="ps_z", bufs=2, space="PSUM"))
    ps_p1 = ctx.enter_context(tc.tile_pool(name="ps_p1", bufs=1, space="PSUM"))
    ps_acc = ctx.enter_context(tc.tile_pool(name="ps_acc", bufs=1, space="PSUM"))

    # ---- constants / inputs resident in SBUF ----
    # tiny weight tensors first so phase 1 isn't stuck behind bulk transfers
    WAcat_sb = consts.tile([128, 36], F32R)
    nc.sync.dma_start(out=WAcat_sb, in_=WAcat_d)
    onehot_sb = consts.tile([4, 4 * 128], F32R)
    nc.sync.dma_start(out=onehot_sb, in_=onehot_d)
    xT_sb = consts.tile([128, BL, N], F32R)
    for b in range(BL):
        nc.sync.dma_start(out=xT_sb[:, b, :], in_=xT_d[b])
    W_sb = consts.tile([128, HD], F32R)
    nc.sync.dma_start(out=W_sb, in_=W_d)
    ones_col = consts.tile([128, 1], BF16)
    nc.vector.memset(ones_col, 1.0)
    maskT_sb = consts.tile([128, NTC, N], BF16)
    nc.sync.dma_start(
        out=maskT_sb,
        in_=maskT_d.rearrange("(jc p) i -> p jc i", p=128))

    # ---- persistent per-batch intermediates ----
    haug_sb = persist.tile([128, BL, NTC, HD], BF16)   # [j-in-chunk, b, jc, h*64+d]
    srow_sb = persist.tile([4, BL, N], F32R)           # raw e_src rows
    Vcol_sb = persist.tile([128, BL, NTC, H], F32)     # exp(e_dst) cols
    wcol_sb = persist.tile([128, BL, NTC, H], F32)     # exp(0.2 e_dst) cols
    U8bc = persist.tile([128, BL, H, N], BF16)         # u8 broadcast over parts

    # ---- phase 1: E = x @ WAcat (rows + cols), haug = x @ W ----
    for b in range(BL):
        # E rows [a=src4+dst4, t] via two 512-col halves (z-pool slots)
        for half in range(2):
            e8 = ps_z.tile([128, 512], F32, tag="z")
            nc.tensor.matmul(e8[0:36, :], lhsT=WAcat_sb,
                             rhs=xT_sb[:, b, half * 512:(half + 1) * 512],
                             start=True, stop=True)
            sl = slice(half * 512, (half + 1) * 512)
            nc.vector.tensor_copy(srow_sb[0:4, b, sl], e8[0:4, :])
        # E cols [t, a] per 128-chunk; exp into V / w columns
        ecol_slot = ps_z.tile([128, 512], F32, tag="z", name=f"ecol_{b}")
        ecol = ecol_slot[:, 0:NTC * 36]
        for tcc in range(NTC):
            nc.tensor.matmul(ecol[:, tcc * 36:(tcc + 1) * 36],
                             lhsT=xT_sb[:, b, tcc * 128:(tcc + 1) * 128],
                             rhs=WAcat_sb, start=True, stop=True)
        dstv = ecol.rearrange("p (t a) -> p t a", t=NTC)[:, :, 32:36]
        nc.scalar.activation(Vcol_sb[:, b], dstv, AF.Exp, bias=0.0, scale=1.0)
        nc.scalar.activation(wcol_sb[:, b], dstv, AF.Exp, bias=0.0, scale=0.2)
        # broadcast e_src row h across partitions via one-hot K=4 matmul,
        # then exp(0.8 x) straight from PSUM into the bf16 U8bc tile
        for h in range(H):
            for half in range(2):
                sl = slice(half * 512, (half + 1) * 512)
                ebc = ps_z.tile([128, 512], F32, tag="z", name=f"ebc_{b}_{h}_{half}")
                nc.tensor.matmul(ebc, lhsT=onehot_sb[:, h * 128:(h + 1) * 128],
                                 rhs=srow_sb[0:4, b, sl], start=True, stop=True)
                nc.scalar.activation(U8bc[:, b, h, sl], ebc, AF.Exp,
                                     bias=0.0, scale=0.8)
        # haug[t, h*64+d] = h in bf16 for the alpha@h contraction
        for tcc in range(NTC):
            hp = ps_p1.tile([128, HD], F32, tag="haug")
            nc.tensor.matmul(hp, lhsT=xT_sb[:, b, tcc * 128:(tcc + 1) * 128],
                             rhs=W_sb, start=True, stop=True)
            nc.scalar.activation(haug_sb[:, b, tcc, :], hp, AF.Copy,
                                 bias=0.0, scale=1.0)

    # ---- phase 2: scores + alpha @ h ----
    for b in range(BL):
        # 4 oacc banks hold the 32 (ic,h) 64-col chains; rs holds row-sums
        obank = [ps_acc.tile([128, 512], F32, tag=f"oacc{k}", name=f"oacc{k}_{b}")
                 for k in range(4)]
        rs = ps_acc.tile([128, 32], F32, tag="rs")
        for jc in range(NTC):
            qwm = qwm_pool.tile([128, H, N], BF16, tag="qwm")
            if (b, jc) in POOL_GROUPS:
                qt = qt_pool.tile([128, H, N], BF16, tag="qt")
                for h in range(H):
                    # u8_i * V_j via ACT copy with column scale
                    nc.scalar.activation(qt[:, h, :], U8bc[:, b, h, :], AF.Copy,
                                         bias=0.0,
                                         scale=Vcol_sb[:, b, jc, h:h + 1])
                    # (u8*V max w) * m on GPSIMD, all SBUF
                    nc.gpsimd.scalar_tensor_tensor(
                        out=qwm[:, h, :], in0=qt[:, h, :],
                        scalar=wcol_sb[:, b, jc, h:h + 1],
                        in1=maskT_sb[:, jc, :],
                        op0=ALU.max, op1=ALU.mult)
            else:
                qt = qt_pool.tile([128, H, N], BF16, tag="qt")
                for h in range(H):
                    nc.vector.tensor_scalar(
                        out=qt[:, h, :], in0=U8bc[:, b, h, :],
                        scalar1=Vcol_sb[:, b, jc, h:h + 1],
                        scalar2=wcol_sb[:, b, jc, h:h + 1],
                        op0=ALU.mult, op1=ALU.max)
                nc.vector.tensor_tensor(
                    out=qwm, in0=qt,
                    in1=maskT_sb[:, jc, :].unsqueeze(1).broadcast_to((128, H, N)),
                    op=ALU.mult)
            # start=True zeroes a whole 2KB psum bank: only the first chain in
            # each bank starts the group, only the last one stops it.
            for h in range(H):
                for ic in range(NTC):
                    c = ic * 4 + h
                    lhsT = qwm[:, h, ic * 128:(ic + 1) * 128]
                    nc.tensor.matmul(
                        obank[c // 8][:, (c % 8) * 64:(c % 8 + 1) * 64],
                        lhsT=lhsT, rhs=haug_sb[:, b, jc, h * 64:(h + 1) * 64],
                        start=(jc == 0 and c % 8 == 0),
                        stop=(jc == NTC - 1 and c % 8 == 7))
                    nc.tensor.matmul(rs[:, c:c + 1], lhsT=lhsT, rhs=ones_col,
                                     start=(jc == 0 and c == 0),
                                     stop=(jc == NTC - 1 and c == 31))
        if taps and b == 0:
            nc.sync.dma_start(out=taps["u8bc"], in_=U8bc[:, 0, 0, :])

            nc.sync.dma_start(out=taps["vcol"], in_=Vcol_sb[:, 0])
            nc.sync.dma_start(out=taps["wcol"], in_=wcol_sb[:, 0])
            rs_tap = osb_pool.tile([128, 32], F32, tag="rstap")
            nc.vector.tensor_copy(rs_tap, rs)
            nc.sync.dma_start(out=taps["rs"], in_=rs_tap)
            nc.sync.dma_start(out=taps["haug"], in_=haug_sb[:, 0])
        rcl = rcl_pool.tile([128, 32], F32, tag="rcl")
        nc.vector.reciprocal(rcl, rs)
        for ic in range(NTC):
            osb = osb_pool.tile([128, H, D], BF16, tag="osb")
            oslice = obank[ic // 2][:, (ic % 2) * 256:(ic % 2 + 1) * 256]
            if b == 0:  # b0 norm on ACT (overlaps b1 scores); b1 on idle-tail DVE
                for h in range(H):
                    nc.scalar.activation(
                        osb[:, h, :], oslice[:, h * 64:(h + 1) * 64], AF.Copy,
                        bias=0.0, scale=rcl[:, ic * 4 + h:ic * 4 + h + 1])
            else:
                nc.vector.tensor_tensor(
                    out=osb, in0=oslice.rearrange("p (h d) -> p h d", h=H),
                    in1=rcl[:, ic * 4:(ic + 1) * 4].unsqueeze(2)
                        .broadcast_to((128, 4, D)), op=ALU.mult)
            nc.sync.dma_start(out=out_d[b, ic * 128:(ic + 1) * 128, :],
                              in_=osb.rearrange("p h d -> p (h d)"))


def prep_inputs(x, adj, W, a_src, a_dst):
    """Host-side prep: shard x over cores, build combined weight layouts."""
    x = np.asarray(x, np.float32)
    adj = np.asarray(adj)
    W = np.asarray(W, np.float32)
    a_src = np.asarray(a_src, np.float32)
    a_dst = np.asarray(a_dst, np.float32)

    maskT = np.ascontiguousarray(adj.T).astype(NPBF)
    Acat = np.zeros((HD, 36), np.float32)
    for h in range(H):
        Acat[h * D:(h + 1) * D, h] = a_src[h]
        Acat[h * D:(h + 1) * D, 32 + h] = a_dst[h]
    WAcat = np.ascontiguousarray(W @ Acat)  # (IN_DIM, 36): src at 0-3, dst at 32-35

    onehot = np.zeros((4, 4 * 128), np.float32)
    for h in range(H):
        onehot[h, h * 128:(h + 1) * 128] = 1.0

    in_maps = []
    for c in range(NCORES):
        xT = np.ascontiguousarray(x[c * BL:(c + 1) * BL].transpose(0, 2, 1))
        in_maps.append({"xT": xT, "W": W, "WAcat": WAcat, "maskT": maskT,
                        "onehot": onehot})
    return in_maps


_PROGRAM_CACHE = {}


def _get_program():
    if "nc" not in _PROGRAM_CACHE:
        _PROGRAM_CACHE["nc"] = build_gat_program()
    return _PROGRAM_CACHE["nc"]


def run_on_hw(inputs, trace=False):
    from concourse.bass_utils import run_bass_kernel_spmd
    nc = _get_program()
    in_maps = prep_inputs(**inputs)
    res = run_bass_kernel_spmd(nc, in_maps, list(range(NCORES)), trace=trace)
    out = np.concatenate(
        [np.asarray(res.results[c]["out"]).astype(np.float32)
         for c in range(NCORES)], axis=0)
    return out, res


def kernel(**inputs) -> np.ndarray:
    out, _ = run_on_hw(inputs, trace=False)
    return out


# revision 33
# speedup vs baseline: 1.0787x; 1.0398x over previous
"""GAT layer kernel for Trainium2 (Bass/Tile), 8-core data-parallel over batch.

Reference (B=16, N=1024, IN_DIM=128, H=4, D=64):
    h = (x @ W).reshape(B,N,H,D)
    e_src/e_dst = einsum('bnhd,hd->bnh', h, a_src/a_dst)
    e[b,i,j,h] = leakyrelu(e_src[b,i,h] + e_dst[b,j,h], 0.2)
    alpha = softmax_j(where(adj[i,j], e, -inf));  out = alpha @ h

Kernel strategy (per core, 2 batches):
  Softmax shift-invariance: with y = s_i + d_j, lrelu(y) = 0.2 s_i + 0.2 d_j
  + 0.8 relu(y); the 0.2 s_i term is constant over j and cancels. So the
  (unnormalized) score reduces to
      PT[j,i] = max(u8_i * V_j, w_j) * m[j,i]
  with u8 = exp(0.8 e_src), V = exp(e_dst), w = exp(0.2 e_dst): one fused DVE
  tensor_scalar (mult, max) per (b,jc,h) against a partition-broadcast u8
  tile, plus one mask multiply shared across the 4 heads. A second path runs
  entirely on PE (rank-1 outer product u8 x V) + GPSIMD (fused (z max w)*m),
  soaking otherwise-idle engines. Row-sums ride separate 1-column matmuls;
  normalization is a batched reciprocal + broadcast multiply.
  All heavy matmuls use bf16 or fp32r (1 PE cycle/row vs 4 for fp32).
"""

import os
import sys
from contextlib import ExitStack

import numpy as np
import ml_dtypes

for _p in ("/opt/trn_rl_repo", "/root/.axon_site/_ro/trn_rl_repo"):
    if os.path.isdir(_p) and _p not in sys.path:
        sys.path.insert(0, _p)

import concourse.bass as bass
import concourse.mybir as mybir
import concourse.tile as tile

F32 = mybir.dt.float32
F32R = mybir.dt.float32r
BF16 = mybir.dt.bfloat16
AF = mybir.ActivationFunctionType
ALU = mybir.AluOpType
NPBF = ml_dtypes.bfloat16

B, N, IN_DIM, H, D = 16, 1024, 128, 4, 64
HD = H * D            # 256
NCORES = 8
BL = B // NCORES      # 2 batches per core
NTC = N // 128        # 8 chunks of 128

# (b, jc) score groups handled by the PE-outer-product + GPSIMD path;
# the rest go through the DVE tensor_scalar path.
POOL_GROUPS = set()
DEBUG_TAPS = False


def _split_excess_waits(nc, max_waits=1):
    """Walrus codegen rejects compute instructions carrying more than one
    sync wait. Move the extras onto engine-matched NoOps inserted
    immediately before the instruction."""
    def _steal_nop(engine):
        engine.nop()
        for fn in nc.m.functions:
            for blk in fn.blocks:
                il = blk.instructions
                if il and type(il[-1]).__name__ == "InstNoOp":
                    nop = il[-1]
                    blk.instructions = il[:-1]
                    return nop
        raise RuntimeError("could not locate appended nop")

    for fn in nc.m.functions:
        for blk in fn.blocks:
            il = list(blk.instructions)
            out = []
            changed = False
            for inst in il:
                si = inst.sync_info
                if (type(inst).__name__ != "InstNoOp" and si is not None
                        and len(si.on_wait) > max_waits):
                    waits = list(si.on_wait)
                    for w in waits[max_waits:]:
                        nop = _steal_nop(nc.engines[inst.engine])
                        nop.sync_info = mybir.SyncInfo(on_wait=[w], on_update=[])
                        out.append(nop)
                    inst.sync_info = mybir.SyncInfo(
                        on_wait=waits[:max_waits], on_update=list(si.on_update))
                    changed = True
                out.append(inst)
            if changed:
                blk.instructions = out


def build_gat_program():
    nc = bass.Bass("TRN2", target_bir_lowering=False, debug=False)
    xT_d = nc.dram_tensor("xT", (BL, IN_DIM, N), F32R, kind="ExternalInput").ap()
    W_d = nc.dram_tensor("W", (IN_DIM, HD), F32R, kind="ExternalInput").ap()
    WAcat_d = nc.dram_tensor("WAcat", (IN_DIM, 36), F32R, kind="ExternalInput").ap()
    maskT_d = nc.dram_tensor("maskT", (N, N), BF16, kind="ExternalInput").ap()
    onehot_d = nc.dram_tensor("onehot", (4, 4 * 128), F32R, kind="ExternalInput").ap()
    out_d = nc.dram_tensor("out", (BL, N, HD), BF16, kind="ExternalOutput").ap()
    taps = {}
    if DEBUG_TAPS:
        taps["u8bc"] = nc.dram_tensor("t_u8bc", (128, N), BF16, kind="ExternalOutput").ap()

        taps["vcol"] = nc.dram_tensor("t_vcol", (128, NTC, H), F32, kind="ExternalOutput").ap()
        taps["wcol"] = nc.dram_tensor("t_wcol", (128, NTC, H), F32, kind="ExternalOutput").ap()
        taps["qwm"] = nc.dram_tensor("t_qwm", (128, H, N), BF16, kind="ExternalOutput").ap()
        taps["rs"] = nc.dram_tensor("t_rs", (128, 32), F32, kind="ExternalOutput").ap()
        taps["haug"] = nc.dram_tensor("t_haug", (128, NTC, HD), BF16, kind="ExternalOutput").ap()

    with tile.TileContext(nc) as tc:
        with ExitStack() as ctx:
            _gat_body(ctx, tc, out_d, xT_d, W_d, WAcat_d, maskT_d, onehot_d,
                      taps)
    _split_excess_waits(nc)
    return nc


def _gat_body(ctx, tc, out_d, xT_d, W_d, WAcat_d, maskT_d, onehot_d, taps=None):
    nc = tc.nc

    consts = ctx.enter_context(tc.tile_pool(name="consts", bufs=1))
    persist = ctx.enter_context(tc.tile_pool(name="persist", bufs=1))
    qt_pool = ctx.enter_context(tc.tile_pool(name="qt", bufs=4))
    qwm_pool = ctx.enter_context(tc.tile_pool(name="qwm", bufs=6))
    osb_pool = ctx.enter_context(tc.tile_pool(name="osb", bufs=3))
    rcl_pool = ctx.enter_context(tc.tile_pool(name="rcl", bufs=3))
    ps_z = ctx.enter_context(tc.tile_pool(name="ps_z", bufs=2, space="PSUM"))
    ps_p1 = ctx.enter_context(tc.tile_pool(name="ps_p1", bufs=1, space="PSUM"))
    ps_acc = ctx.enter_context(tc.tile_pool(name="ps_acc", bufs=1, space="PSUM"))

    # ---- constants / inputs resident in SBUF ----
    # tiny weight tensors first so phase 1 isn't stuck behind bulk transfers
    WAcat_sb = consts.tile([128, 36], F32R)
    nc.sync.dma_start(out=WAcat_sb, in_=WAcat_d)
    onehot_sb = consts.tile([4, 4 * 128], F32R)
    nc.sync.dma_start(out=onehot_sb, in_=onehot_d)
    xT_sb = consts.tile([128, BL, N], F32R)
    for b in range(BL):
        nc.sync.dma_start(out=xT_sb[:, b, :], in_=xT_d[b])
    W_sb = consts.tile([128, HD], F32R)
    nc.sync.dma_start(out=W_sb, in_=W_d)
    ones_col = consts.tile([128, 1], BF16)
    nc.vector.memset(ones_col, 1.0)
    maskT_sb = consts.tile([128, NTC, N], BF16)
    nc.sync.dma_start(
        out=maskT_sb,
        in_=maskT_d.rearrange("(jc p) i -> p jc i", p=128))

    # ---- persistent per-batch intermediates ----
    haug_sb = persist.tile([128, BL, NTC, HD], BF16)   # [j-in-chunk, b, jc, h*64+d]
    srow_sb = persist.tile([4, BL, N], F32R)           # raw e_src rows
    Vcol_sb = persist.tile([128, BL, NTC, H], F32)     # exp(e_dst) cols
    wcol_sb = persist.tile([128, BL, NTC, H], F32)     # exp(0.2 e_dst) cols
    U8bc = persist.tile([128, BL, H, N], BF16)         # u8 broadcast over parts

    # ---- phase 1: E = x @ WAcat (rows + cols), haug = x @ W ----
    for b in range(BL):
        # E rows [a=src4+dst4, t] via two 512-col halves (z-pool slots)
        for half in range(2):
            e8 = ps_z.tile([128, 512], F32, tag="z")
            nc.tensor.matmul(e8[0:36, :], lhsT=WAcat_sb,
                             rhs=xT_sb[:, b, half * 512:(half + 1) * 512],
                             start=True, stop=True)
            sl = slice(half * 512, (half + 1) * 512)
            nc.vector.tensor_copy(srow_sb[0:4, b, sl], e8[0:4, :])
        # E cols [t, a] per 128-chunk; exp into V / w columns
        ecol_slot = ps_z.tile([128, 512], F32, tag="z", name=f"ecol_{b}")
        ecol = ecol_slot[:, 0:NTC * 36]
        for tcc in range(NTC):
            nc.tensor.matmul(ecol[:, tcc * 36:(tcc + 1) * 36],
                             lhsT=xT_sb[:, b, tcc * 128:(tcc + 1) * 128],
                             rhs=WAcat_sb, start=True, stop=True)
        dstv = ecol.rearrange("p (t a) -> p t a", t=NTC)[:, :, 32:36]
        nc.scalar.activation(Vcol_sb[:, b], dstv, AF.Exp, bias=0.0, scale=1.0)
        nc.scalar.activation(wcol_sb[:, b], dstv, AF.Exp, bias=0.0, scale=0.2)
        # broadcast e_src row h across partitions via one-hot K=4 matmul,
        # then exp(0.8 x) straight from PSUM into the bf16 U8bc tile
        for h in range(H):
            for half in range(2):
                sl = slice(half * 512, (half + 1) * 512)
                ebc = ps_z.tile([128, 512], F32, tag="z", name=f"ebc_{b}_{h}_{half}")
                nc.tensor.matmul(ebc, lhsT=onehot_sb[:, h * 128:(h + 1) * 128],
                                 rhs=srow_sb[0:4, b, sl], start=True, stop=True)
                nc.scalar.activation(U8bc[:, b, h, sl], ebc, AF.Exp,
                                     bias=0.0, scale=0.8)
        # haug[t, h*64+d] = h in bf16 for the alpha@h contraction
        for tcc in range(NTC):
            hp = ps_p1.tile([128, HD], F32, tag="haug")
            nc.tensor.matmul(hp, lhsT=xT_sb[:, b, tcc * 128:(tcc + 1) * 128],
                             rhs=W_sb, start=True, stop=True)
            nc.scalar.activation(haug_sb[:, b, tcc, :], hp, AF.Copy,
                                 bias=0.0, scale=1.0)

    # ---- phase 2: scores + alpha @ h ----
    for b in range(BL):
        # 4 oacc banks hold the 32 (ic,h) 64-col chains; rs holds row-sums
        obank = [ps_acc.tile([128, 512], F32, tag=f"oacc{k}", name=f"oacc{k}_{b}")
                 for k in range(4)]
        rs = ps_acc.tile([128, 32], F32, tag="rs")
        for jc in range(NTC):
            qwm = qwm_pool.tile([128, H, N], BF16, tag="qwm")
            if (b, jc) in POOL_GROUPS:
                qt = qt_pool.tile([128, H, N], BF16, tag="qt")
                for h in range(H):
                    # u8_i * V_j via ACT copy with column scale
                    nc.scalar.activation(qt[:, h, :], U8bc[:, b, h, :], AF.Copy,
                                         bias=0.0,
                                         scale=Vcol_sb[:, b, jc, h:h + 1])
                    # (u8*V max w) * m on GPSIMD, all SBUF
                    nc.gpsimd.scalar_tensor_tensor(
                        out=qwm[:, h, :], in0=qt[:, h, :],
                        scalar=wcol_sb[:, b, jc, h:h + 1],
                        in1=maskT_sb[:, jc, :],
                        op0=ALU.max, op1=ALU.mult)
            else:
                qt = qt_pool.tile([128, H, N], BF16, tag="qt")
                for h in range(H):
                    nc.vector.tensor_scalar(
                        out=qt[:, h, :], in0=U8bc[:, b, h, :],
                        scalar1=Vcol_sb[:, b, jc, h:h + 1],
                        scalar2=wcol_sb[:, b, jc, h:h + 1],
                        op0=ALU.mult, op1=ALU.max)
                nc.vector.tensor_tensor(
                    out=qwm, in0=qt,
                    in1=maskT_sb[:, jc, :].unsqueeze(1).broadcast_to((128, H, N)),
                    op=ALU.mult)
            # start=True zeroes a whole 2KB psum bank: only the first chain in
            # each bank starts the group, only the last one stops it.
            for h in range(H):
                for ic in range(NTC):
                    c = ic * 4 + h
                    lhsT = qwm[:, h, ic * 128:(ic + 1) * 128]
                    nc.tensor.matmul(
                        obank[c // 8][:, (c % 8) * 64:(c % 8 + 1) * 64],
                        lhsT=lhsT, rhs=haug_sb[:, b, jc, h * 64:(h + 1) * 64],
                        start=(jc == 0 and c % 8 == 0),
                        stop=(jc == NTC - 1 and c % 8 == 7))
                    nc.tensor.matmul(rs[:, c:c + 1], lhsT=lhsT, rhs=ones_col,
                                     start=(jc == 0 and c == 0),
                                     stop=(jc == NTC - 1 and c == 31))
        if taps and b == 0:
            nc.sync.dma_start(out=taps["u8bc"], in_=U8bc[:, 0, 0, :])

            nc.sync.dma_start(out=taps["vcol"], in_=Vcol_sb[:, 0])
            nc.sync.dma_start(out=taps["wcol"], in_=wcol_sb[:, 0])
            rs_tap = osb_pool.tile([128, 32], F32, tag="rstap")
            nc.vector.tensor_copy(rs_tap, rs)
            nc.sync.dma_start(out=taps["rs"], in_=rs_tap)
            nc.sync.dma_start(out=taps["haug"], in_=haug_sb[:, 0])
        rcl = rcl_pool.tile([128, 32], F32, tag="rcl")
        nc.vector.reciprocal(rcl, rs)
        osb = osb_pool.tile([128, NTC, HD], BF16, tag="osb")
        for ic in range(NTC):
            oslice = obank[ic // 2][:, (ic % 2) * 256:(ic % 2 + 1) * 256]
            if b == 0:  # b0 norm on ACT (overlaps b1 scores); b1 on idle-tail DVE
                for h in range(H):
                    nc.scalar.activation(
                        osb[:, ic, h * 64:(h + 1) * 64],
                        oslice[:, h * 64:(h + 1) * 64], AF.Copy,
                        bias=0.0, scale=rcl[:, ic * 4 + h:ic * 4 + h + 1])
            else:
                nc.vector.tensor_tensor(
                    out=osb[:, ic, :].rearrange("p (h d) -> p h d", h=H),
                    in0=oslice.rearrange("p (h d) -> p h d", h=H),
                    in1=rcl[:, ic * 4:(ic + 1) * 4].unsqueeze(2)
                        .broadcast_to((128, 4, D)), op=ALU.mult)
        nc.sync.dma_start(
            out=out_d[b].rearrange("(ic p) d -> p ic d", p=128), in_=osb)


def prep_inputs(x, adj, W, a_src, a_dst):
    """Host-side prep: shard x over cores, build combined weight layouts."""
    x = np.asarray(x, np.float32)
    adj = np.asarray(adj)
    W = np.asarray(W, np.float32)
    a_src = np.asarray(a_src, np.float32)
    a_dst = np.asarray(a_dst, np.float32)

    maskT = np.ascontiguousarray(adj.T).astype(NPBF)
    Acat = np.zeros((HD, 36), np.float32)
    for h in range(H):
        Acat[h * D:(h + 1) * D, h] = a_src[h]
        Acat[h * D:(h + 1) * D, 32 + h] = a_dst[h]
    WAcat = np.ascontiguousarray(W @ Acat)  # (IN_DIM, 36): src at 0-3, dst at 32-35

    onehot = np.zeros((4, 4 * 128), np.float32)
    for h in range(H):
        onehot[h, h * 128:(h + 1) * 128] = 1.0

    in_maps = []
    for c in range(NCORES):
        xT = np.ascontiguousarray(x[c * BL:(c + 1) * BL].transpose(0, 2, 1))
        in_maps.append({"xT": xT, "W": W, "WAcat": WAcat, "maskT": maskT,
                        "onehot": onehot})
    return in_maps


_PROGRAM_CACHE = {}


def _get_program():
    if "nc" not in _PROGRAM_CACHE:
        _PROGRAM_CACHE["nc"] = build_gat_program()
    return _PROGRAM_CACHE["nc"]


def run_on_hw(inputs, trace=False):
    from concourse.bass_utils import run_bass_kernel_spmd
    nc = _get_program()
    in_maps = prep_inputs(**inputs)
    res = run_bass_kernel_spmd(nc, in_maps, list(range(NCORES)), trace=trace)
    out = np.concatenate(
        [np.asarray(res.results[c]["out"]).astype(np.float32)
         for c in range(NCORES)], axis=0)
    return out, res


def kernel(**inputs) -> np.ndarray:
    out, _ = run_on_hw(inputs, trace=False)
    return out


# revision 34
# speedup vs baseline: 1.0955x; 1.0156x over previous
"""GAT layer kernel for Trainium2 (Bass/Tile), 8-core data-parallel over batch.

Reference (B=16, N=1024, IN_DIM=128, H=4, D=64):
    h = (x @ W).reshape(B,N,H,D)
    e_src/e_dst = einsum('bnhd,hd->bnh', h, a_src/a_dst)
    e[b,i,j,h] = leakyrelu(e_src[b,i,h] + e_dst[b,j,h], 0.2)
    alpha = softmax_j(where(adj[i,j], e, -inf));  out = alpha @ h

Kernel strategy (per core, 2 batches):
  Softmax shift-invariance: with y = s_i + d_j, lrelu(y) = 0.2 s_i + 0.2 d_j
  + 0.8 relu(y); the 0.2 s_i term is constant over j and cancels. So the
  (unnormalized) score reduces to
      PT[j,i] = max(u8_i * V_j, w_j) * m[j,i]
  with u8 = exp(0.8 e_src), V = exp(e_dst), w = exp(0.2 e_dst): one fused DVE
  tensor_scalar (mult, max) per (b,jc,h) against a partition-broadcast u8
  tile, plus one mask multiply shared across the 4 heads. A second path runs
  entirely on PE (rank-1 outer product u8 x V) + GPSIMD (fused (z max w)*m),
  soaking otherwise-idle engines. Row-sums ride separate 1-column matmuls;
  normalization is a batched reciprocal + broadcast multiply.
  All heavy matmuls use bf16 or fp32r (1 PE cycle/row vs 4 for fp32).
"""

import os
import sys
from contextlib import ExitStack

import numpy as np
import ml_dtypes

for _p in ("/opt/trn_rl_repo", "/root/.axon_site/_ro/trn_rl_repo"):
    if os.path.isdir(_p) and _p not in sys.path:
        sys.path.insert(0, _p)

import concourse.bass as bass
import concourse.mybir as mybir
import concourse.tile as tile

F32 = mybir.dt.float32
F32R = mybir.dt.float32r
BF16 = mybir.dt.bfloat16
AF = mybir.ActivationFunctionType
ALU = mybir.AluOpType
NPBF = ml_dtypes.bfloat16

B, N, IN_DIM, H, D = 16, 1024, 128, 4, 64
HD = H * D            # 256
NCORES = 8
BL = B // NCORES      # 2 batches per core
NTC = N // 128        # 8 chunks of 128

# (b, jc) score groups handled by the PE-outer-product + GPSIMD path;
# the rest go through the DVE tensor_scalar path.
POOL_GROUPS = set()
DEBUG_TAPS = False


def _split_excess_waits(nc, max_waits=1):
    """Walrus codegen rejects compute instructions carrying more than one
    sync wait. Move the extras onto engine-matched NoOps inserted
    immediately before the instruction."""
    def _steal_nop(engine):
        engine.nop()
        for fn in nc.m.functions:
            for blk in fn.blocks:
                il = blk.instructions
                if il and type(il[-1]).__name__ == "InstNoOp":
                    nop = il[-1]
                    blk.instructions = il[:-1]
                    return nop
        raise RuntimeError("could not locate appended nop")

    for fn in nc.m.functions:
        for blk in fn.blocks:
            il = list(blk.instructions)
            out = []
            changed = False
            for inst in il:
                si = inst.sync_info
                if (type(inst).__name__ != "InstNoOp" and si is not None
                        and len(si.on_wait) > max_waits):
                    waits = list(si.on_wait)
                    for w in waits[max_waits:]:
                        nop = _steal_nop(nc.engines[inst.engine])
                        nop.sync_info = mybir.SyncInfo(on_wait=[w], on_update=[])
                        out.append(nop)
                    inst.sync_info = mybir.SyncInfo(
                        on_wait=waits[:max_waits], on_update=list(si.on_update))
                    changed = True
                out.append(inst)
            if changed:
                blk.instructions = out


def build_gat_program():
    nc = bass.Bass("TRN2", target_bir_lowering=False, debug=False)
    xT_d = nc.dram_tensor("xT", (BL, IN_DIM, N), F32R, kind="ExternalInput").ap()
    W_d = nc.dram_tensor("W", (IN_DIM, HD), F32R, kind="ExternalInput").ap()
    WAcat_d = nc.dram_tensor("WAcat", (IN_DIM, 36), F32R, kind="ExternalInput").ap()
    maskT_d = nc.dram_tensor("maskT", (N, N), BF16, kind="ExternalInput").ap()
    onehot_d = nc.dram_tensor("onehot", (4, 4 * 128), F32R, kind="ExternalInput").ap()
    out_d = nc.dram_tensor("out", (BL, N, HD), BF16, kind="ExternalOutput").ap()
    taps = {}
    if DEBUG_TAPS:
        taps["u8bc"] = nc.dram_tensor("t_u8bc", (128, N), BF16, kind="ExternalOutput").ap()

        taps["vcol"] = nc.dram_tensor("t_vcol", (128, NTC, H), F32, kind="ExternalOutput").ap()
        taps["wcol"] = nc.dram_tensor("t_wcol", (128, NTC, H), F32, kind="ExternalOutput").ap()
        taps["qwm"] = nc.dram_tensor("t_qwm", (128, H, N), BF16, kind="ExternalOutput").ap()
        taps["rs"] = nc.dram_tensor("t_rs", (128, 32), F32, kind="ExternalOutput").ap()
        taps["haug"] = nc.dram_tensor("t_haug", (128, NTC, HD), BF16, kind="ExternalOutput").ap()

    with tile.TileContext(nc) as tc:
        with ExitStack() as ctx:
            _gat_body(ctx, tc, out_d, xT_d, W_d, WAcat_d, maskT_d, onehot_d,
                      taps)
    _split_excess_waits(nc)
    return nc


def _gat_body(ctx, tc, out_d, xT_d, W_d, WAcat_d, maskT_d, onehot_d, taps=None):
    nc = tc.nc

    consts = ctx.enter_context(tc.tile_pool(name="consts", bufs=1))
    persist = ctx.enter_context(tc.tile_pool(name="persist", bufs=1))
    qt_pool = ctx.enter_context(tc.tile_pool(name="qt", bufs=4))
    qwm_pool = ctx.enter_context(tc.tile_pool(name="qwm", bufs=6))
    osb_pool = ctx.enter_context(tc.tile_pool(name="osb", bufs=3))
    rcl_pool = ctx.enter_context(tc.tile_pool(name="rcl", bufs=3))
    ps_z = ctx.enter_context(tc.tile_pool(name="ps_z", bufs=2, space="PSUM"))
    ps_p1 = ctx.enter_context(tc.tile_pool(name="ps_p1", bufs=1, space="PSUM"))
    ps_acc = ctx.enter_context(tc.tile_pool(name="ps_acc", bufs=1, space="PSUM"))

    # ---- constants / inputs resident in SBUF ----
    # xT b0 first: its descriptor-gen overlaps the tiny weight transfers
    xT_sb = consts.tile([128, BL, N], F32R)
    nc.sync.dma_start(out=xT_sb[:, 0, :], in_=xT_d[0])
    WAcat_sb = consts.tile([128, 36], F32R)
    nc.sync.dma_start(out=WAcat_sb, in_=WAcat_d)
    onehot_sb = consts.tile([4, 4 * 128], F32R)
    nc.sync.dma_start(out=onehot_sb, in_=onehot_d)
    nc.sync.dma_start(out=xT_sb[:, 1, :], in_=xT_d[1])
    W_sb = consts.tile([128, HD], F32R)
    nc.sync.dma_start(out=W_sb, in_=W_d)
    ones_col = consts.tile([128, 1], BF16)
    nc.vector.memset(ones_col, 1.0)
    maskT_sb = consts.tile([128, NTC, N], BF16)
    nc.sync.dma_start(
        out=maskT_sb,
        in_=maskT_d.rearrange("(jc p) i -> p jc i", p=128))

    # ---- persistent per-batch intermediates ----
    haug_sb = persist.tile([128, BL, NTC, HD], BF16)   # [j-in-chunk, b, jc, h*64+d]
    srow_sb = persist.tile([4, BL, N], F32R)           # raw e_src rows
    Vcol_sb = persist.tile([128, BL, NTC, H], F32)     # exp(e_dst) cols
    wcol_sb = persist.tile([128, BL, NTC, H], F32)     # exp(0.2 e_dst) cols
    U8bc = persist.tile([128, BL, H, N], BF16)         # u8 broadcast over parts

    # ---- phase 1: E = x @ WAcat (rows + cols), haug = x @ W ----
    for b in range(BL):
        # E rows [a=src4+dst4, t] via two 512-col halves (z-pool slots)
        for half in range(2):
            e8 = ps_z.tile([128, 512], F32, tag="z")
            nc.tensor.matmul(e8[0:36, :], lhsT=WAcat_sb,
                             rhs=xT_sb[:, b, half * 512:(half + 1) * 512],
                             start=True, stop=True)
            sl = slice(half * 512, (half + 1) * 512)
            nc.vector.tensor_copy(srow_sb[0:4, b, sl], e8[0:4, :])
        # E cols [t, a] per 128-chunk; exp into V / w columns
        ecol_slot = ps_z.tile([128, 512], F32, tag="z", name=f"ecol_{b}")
        ecol = ecol_slot[:, 0:NTC * 36]
        for tcc in range(NTC):
            nc.tensor.matmul(ecol[:, tcc * 36:(tcc + 1) * 36],
                             lhsT=xT_sb[:, b, tcc * 128:(tcc + 1) * 128],
                             rhs=WAcat_sb, start=True, stop=True)
        dstv = ecol.rearrange("p (t a) -> p t a", t=NTC)[:, :, 32:36]
        nc.scalar.activation(Vcol_sb[:, b], dstv, AF.Exp, bias=0.0, scale=1.0)
        nc.scalar.activation(wcol_sb[:, b], dstv, AF.Exp, bias=0.0, scale=0.2)
        # broadcast e_src row h across partitions via one-hot K=4 matmul,
        # then exp(0.8 x) straight from PSUM into the bf16 U8bc tile
        for h in range(H):
            for half in range(2):
                sl = slice(half * 512, (half + 1) * 512)
                ebc = ps_z.tile([128, 512], F32, tag="z", name=f"ebc_{b}_{h}_{half}")
                nc.tensor.matmul(ebc, lhsT=onehot_sb[:, h * 128:(h + 1) * 128],
                                 rhs=srow_sb[0:4, b, sl], start=True, stop=True)
                nc.scalar.activation(U8bc[:, b, h, sl], ebc, AF.Exp,
                                     bias=0.0, scale=0.8)
        # haug[t, h*64+d] = h in bf16 for the alpha@h contraction
        for tcc in range(NTC):
            hp = ps_p1.tile([128, HD], F32, tag="haug")
            nc.tensor.matmul(hp, lhsT=xT_sb[:, b, tcc * 128:(tcc + 1) * 128],
                             rhs=W_sb, start=True, stop=True)
            nc.scalar.activation(haug_sb[:, b, tcc, :], hp, AF.Copy,
                                 bias=0.0, scale=1.0)

    # ---- phase 2: scores + alpha @ h ----
    for b in range(BL):
        # 4 oacc banks hold the 32 (ic,h) 64-col chains; rs holds row-sums
        obank = [ps_acc.tile([128, 512], F32, tag=f"oacc{k}", name=f"oacc{k}_{b}")
                 for k in range(4)]
        rs = ps_acc.tile([128, 32], F32, tag="rs")
        for jc in range(NTC):
            qwm = qwm_pool.tile([128, H, N], BF16, tag="qwm")
            if (b, jc) in POOL_GROUPS:
                qt = qt_pool.tile([128, H, N], BF16, tag="qt")
                for h in range(H):
                    # u8_i * V_j via ACT copy with column scale
                    nc.scalar.activation(qt[:, h, :], U8bc[:, b, h, :], AF.Copy,
                                         bias=0.0,
                                         scale=Vcol_sb[:, b, jc, h:h + 1])
                    # (u8*V max w) * m on GPSIMD, all SBUF
                    nc.gpsimd.scalar_tensor_tensor(
                        out=qwm[:, h, :], in0=qt[:, h, :],
                        scalar=wcol_sb[:, b, jc, h:h + 1],
                        in1=maskT_sb[:, jc, :],
                        op0=ALU.max, op1=ALU.mult)
            else:
                qt = qt_pool.tile([128, H, N], BF16, tag="qt")
                for h in range(H):
                    nc.vector.tensor_scalar(
                        out=qt[:, h, :], in0=U8bc[:, b, h, :],
                        scalar1=Vcol_sb[:, b, jc, h:h + 1],
                        scalar2=wcol_sb[:, b, jc, h:h + 1],
                        op0=ALU.mult, op1=ALU.max)
                nc.vector.tensor_tensor(
                    out=qwm, in0=qt,
                    in1=maskT_sb[:, jc, :].unsqueeze(1).broadcast_to((128, H, N)),
                    op=ALU.mult)
            # start=True zeroes a whole 2KB psum bank: only the first chain in
            # each bank starts the group, only the last one stops it.
            for h in range(H):
                for ic in range(NTC):
                    c = ic * 4 + h
                    lhsT = qwm[:, h, ic * 128:(ic + 1) * 128]
                    nc.tensor.matmul(
                        obank[c // 8][:, (c % 8) * 64:(c % 8 + 1) * 64],
                        lhsT=lhsT, rhs=haug_sb[:, b, jc, h * 64:(h + 1) * 64],
                        start=(jc == 0 and c % 8 == 0),
                        stop=(jc == NTC - 1 and c % 8 == 7))
                    nc.tensor.matmul(rs[:, c:c + 1], lhsT=lhsT, rhs=ones_col,
                                     start=(jc == 0 and c == 0),
                                     stop=(jc == NTC - 1 and c == 31))
        if taps and b == 0:
            nc.sync.dma_start(out=taps["u8bc"], in_=U8bc[:, 0, 0, :])

            nc.sync.dma_start(out=taps["vcol"], in_=Vcol_sb[:, 0])
            nc.sync.dma_start(out=taps["wcol"], in_=wcol_sb[:, 0])
            rs_tap = osb_pool.tile([128, 32], F32, tag="rstap")
            nc.vector.tensor_copy(rs_tap, rs)
            nc.sync.dma_start(out=taps["rs"], in_=rs_tap)
            nc.sync.dma_start(out=taps["haug"], in_=haug_sb[:, 0])
        rcl = rcl_pool.tile([128, 32], F32, tag="rcl")
        nc.vector.reciprocal(rcl, rs)
        osb = osb_pool.tile([128, NTC, HD], BF16, tag="osb")
        for ic in range(NTC):
            oslice = obank[ic // 2][:, (ic % 2) * 256:(ic % 2 + 1) * 256]
            if b == 0:  # b0 norm on ACT (overlaps b1 scores); b1 on idle-tail DVE
                for h in range(H):
                    nc.scalar.activation(
                        osb[:, ic, h * 64:(h + 1) * 64],
                        oslice[:, h * 64:(h + 1) * 64], AF.Copy,
                        bias=0.0, scale=rcl[:, ic * 4 + h:ic * 4 + h + 1])
            else:
                nc.vector.tensor_tensor(
                    out=osb[:, ic, :].rearrange("p (h d) -> p h d", h=H),
                    in0=oslice.rearrange("p (h d) -> p h d", h=H),
                    in1=rcl[:, ic * 4:(ic + 1) * 4].unsqueeze(2)
                        .broadcast_to((128, 4, D)), op=ALU.mult)
        nc.sync.dma_start(
            out=out_d[b].rearrange("(ic p) d -> p ic d", p=128), in_=osb)


def prep_inputs(x, adj, W, a_src, a_dst):
    """Host-side prep: shard x over cores, build combined weight layouts."""
    x = np.asarray(x, np.float32)
    adj = np.asarray(adj)
    W = np.asarray(W, np.float32)
    a_src = np.asarray(a_src, np.float32)
    a_dst = np.asarray(a_dst, np.float32)

    maskT = np.ascontiguousarray(adj.T).astype(NPBF)
    Acat = np.zeros((HD, 36), np.float32)
    for h in range(H):
        Acat[h * D:(h + 1) * D, h] = a_src[h]
        Acat[h * D:(h + 1) * D, 32 + h] = a_dst[h]
    WAcat = np.ascontiguousarray(W @ Acat)  # (IN_DIM, 36): src at 0-3, dst at 32-35

    onehot = np.zeros((4, 4 * 128), np.float32)
    for h in range(H):
        onehot[h, h * 128:(h + 1) * 128] = 1.0

    in_maps = []
    for c in range(NCORES):
        xT = np.ascontiguousarray(x[c * BL:(c + 1) * BL].transpose(0, 2, 1))
        in_maps.append({"xT": xT, "W": W, "WAcat": WAcat, "maskT": maskT,
                        "onehot": onehot})
    return in_maps


_PROGRAM_CACHE = {}


def _get_program():
    if "nc" not in _PROGRAM_CACHE:
        _PROGRAM_CACHE["nc"] = build_gat_program()
    return _PROGRAM_CACHE["nc"]


def run_on_hw(inputs, trace=False):
    from concourse.bass_utils import run_bass_kernel_spmd
    nc = _get_program()
    in_maps = prep_inputs(**inputs)
    res = run_bass_kernel_spmd(nc, in_maps, list(range(NCORES)), trace=trace)
    out = np.concatenate(
        [np.asarray(res.results[c]["out"]).astype(np.float32)
         for c in range(NCORES)], axis=0)
    return out, res


def kernel(**inputs) -> np.ndarray:
    out, _ = run_on_hw(inputs, trace=False)
    return out


# revision 35
# speedup vs baseline: 1.1186x; 1.0211x over previous
"""GAT layer kernel for Trainium2 (Bass/Tile), 8-core data-parallel over batch.

Reference (B=16, N=1024, IN_DIM=128, H=4, D=64):
    h = (x @ W).reshape(B,N,H,D)
    e_src/e_dst = einsum('bnhd,hd->bnh', h, a_src/a_dst)
    e[b,i,j,h] = leakyrelu(e_src[b,i,h] + e_dst[b,j,h], 0.2)
    alpha = softmax_j(where(adj[i,j], e, -inf));  out = alpha @ h

Kernel strategy (per core, 2 batches):
  Softmax shift-invariance: with y = s_i + d_j, lrelu(y) = 0.2 s_i + 0.2 d_j
  + 0.8 relu(y); the 0.2 s_i term is constant over j and cancels. So the
  (unnormalized) score reduces to
      PT[j,i] = max(u8_i * V_j, w_j) * m[j,i]
  with u8 = exp(0.8 e_src), V = exp(e_dst), w = exp(0.2 e_dst): one fused DVE
  tensor_scalar (mult, max) per (b,jc,h) against a partition-broadcast u8
  tile, plus one mask multiply shared across the 4 heads. A second path runs
  entirely on PE (rank-1 outer product u8 x V) + GPSIMD (fused (z max w)*m),
  soaking otherwise-idle engines. Row-sums ride separate 1-column matmuls;
  normalization is a batched reciprocal + broadcast multiply.
  All heavy matmuls use bf16 or fp32r (1 PE cycle/row vs 4 for fp32).
"""

import os
import sys
from contextlib import ExitStack

import numpy as np
import ml_dtypes

for _p in ("/opt/trn_rl_repo", "/root/.axon_site/_ro/trn_rl_repo"):
    if os.path.isdir(_p) and _p not in sys.path:
        sys.path.insert(0, _p)

import concourse.bass as bass
import concourse.mybir as mybir
import concourse.tile as tile

F32 = mybir.dt.float32
F32R = mybir.dt.float32r
BF16 = mybir.dt.bfloat16
AF = mybir.ActivationFunctionType
ALU = mybir.AluOpType
NPBF = ml_dtypes.bfloat16

B, N, IN_DIM, H, D = 16, 1024, 128, 4, 64
HD = H * D            # 256
NCORES = 8
BL = B // NCORES      # 2 batches per core
NTC = N // 128        # 8 chunks of 128

# (b, jc) score groups handled by the PE-outer-product + GPSIMD path;
# the rest go through the DVE tensor_scalar path.
POOL_GROUPS = set()
DEBUG_TAPS = False


def _split_excess_waits(nc, max_waits=1):
    """Walrus codegen rejects compute instructions carrying more than one
    sync wait. Move the extras onto engine-matched NoOps inserted
    immediately before the instruction."""
    def _steal_nop(engine):
        engine.nop()
        for fn in nc.m.functions:
            for blk in fn.blocks:
                il = blk.instructions
                if il and type(il[-1]).__name__ == "InstNoOp":
                    nop = il[-1]
                    blk.instructions = il[:-1]
                    return nop
        raise RuntimeError("could not locate appended nop")

    for fn in nc.m.functions:
        for blk in fn.blocks:
            il = list(blk.instructions)
            out = []
            changed = False
            for inst in il:
                si = inst.sync_info
                if (type(inst).__name__ != "InstNoOp" and si is not None
                        and len(si.on_wait) > max_waits):
                    waits = list(si.on_wait)
                    for w in waits[max_waits:]:
                        nop = _steal_nop(nc.engines[inst.engine])
                        nop.sync_info = mybir.SyncInfo(on_wait=[w], on_update=[])
                        out.append(nop)
                    inst.sync_info = mybir.SyncInfo(
                        on_wait=waits[:max_waits], on_update=list(si.on_update))
                    changed = True
                out.append(inst)
            if changed:
                blk.instructions = out


def build_gat_program():
    nc = bass.Bass("TRN2", target_bir_lowering=False, debug=False)
    xT_d = nc.dram_tensor("xT", (BL, IN_DIM, N), F32R, kind="ExternalInput").ap()
    W_d = nc.dram_tensor("W", (IN_DIM, HD), F32R, kind="ExternalInput").ap()
    WAcat_d = nc.dram_tensor("WAcat", (IN_DIM, 36), F32R, kind="ExternalInput").ap()
    maskT_d = nc.dram_tensor("maskT", (N, N), BF16, kind="ExternalInput").ap()
    onehot_d = nc.dram_tensor("onehot", (4, 4 * 128), F32R, kind="ExternalInput").ap()
    out_d = nc.dram_tensor("out", (BL, N, HD), BF16, kind="ExternalOutput").ap()
    taps = {}
    if DEBUG_TAPS:
        taps["u8bc"] = nc.dram_tensor("t_u8bc", (128, N), BF16, kind="ExternalOutput").ap()

        taps["vcol"] = nc.dram_tensor("t_vcol", (128, NTC, H), F32, kind="ExternalOutput").ap()
        taps["wcol"] = nc.dram_tensor("t_wcol", (128, NTC, H), F32, kind="ExternalOutput").ap()
        taps["qwm"] = nc.dram_tensor("t_qwm", (128, H, N), BF16, kind="ExternalOutput").ap()
        taps["rs"] = nc.dram_tensor("t_rs", (128, 32), F32, kind="ExternalOutput").ap()
        taps["haug"] = nc.dram_tensor("t_haug", (128, NTC, HD), BF16, kind="ExternalOutput").ap()

    with tile.TileContext(nc) as tc:
        with ExitStack() as ctx:
            _gat_body(ctx, tc, out_d, xT_d, W_d, WAcat_d, maskT_d, onehot_d,
                      taps)
    _split_excess_waits(nc)
    return nc


def _gat_body(ctx, tc, out_d, xT_d, W_d, WAcat_d, maskT_d, onehot_d, taps=None):
    nc = tc.nc

    consts = ctx.enter_context(tc.tile_pool(name="consts", bufs=1))
    persist = ctx.enter_context(tc.tile_pool(name="persist", bufs=1))
    qt_pool = ctx.enter_context(tc.tile_pool(name="qt", bufs=4))
    qwm_pool = ctx.enter_context(tc.tile_pool(name="qwm", bufs=6))
    osb_pool = ctx.enter_context(tc.tile_pool(name="osb", bufs=3))
    rcl_pool = ctx.enter_context(tc.tile_pool(name="rcl", bufs=3))
    ps_z = ctx.enter_context(tc.tile_pool(name="ps_z", bufs=2, space="PSUM"))
    ps_p1 = ctx.enter_context(tc.tile_pool(name="ps_p1", bufs=1, space="PSUM"))
    ps_acc = ctx.enter_context(tc.tile_pool(name="ps_acc", bufs=1, space="PSUM"))

    # ---- constants / inputs resident in SBUF ----
    # xT b0 first: its descriptor-gen overlaps the tiny weight transfers
    xT_sb = consts.tile([128, BL, N], F32R)
    nc.sync.dma_start(out=xT_sb[:, 0, :], in_=xT_d[0])
    WAcat_sb = consts.tile([128, 36], F32R)
    nc.sync.dma_start(out=WAcat_sb, in_=WAcat_d)
    onehot_sb = consts.tile([4, 4 * 128], F32R)
    nc.sync.dma_start(out=onehot_sb, in_=onehot_d)
    nc.sync.dma_start(out=xT_sb[:, 1, :], in_=xT_d[1])
    W_sb = consts.tile([128, HD], F32R)
    nc.sync.dma_start(out=W_sb, in_=W_d)
    ones_col = consts.tile([128, 1], BF16)
    nc.vector.memset(ones_col, 1.0)
    maskT_sb = consts.tile([128, NTC, N], BF16)
    nc.sync.dma_start(
        out=maskT_sb,
        in_=maskT_d.rearrange("(jc p) i -> p jc i", p=128))

    # ---- persistent per-batch intermediates ----
    haug_sb = persist.tile([128, BL, NTC, HD], BF16)   # [j-in-chunk, b, jc, h*64+d]
    srow_sb = persist.tile([4, BL, N], F32R)           # raw e_src rows
    Vcol_sb = persist.tile([128, BL, NTC, H], F32)     # exp(e_dst) cols
    wcol_sb = persist.tile([128, BL, NTC, H], F32)     # exp(0.2 e_dst) cols
    U8bc = persist.tile([128, BL, H, N], BF16)         # u8 broadcast over parts

    # ---- phase 1: E = x @ WAcat (rows + cols), haug = x @ W ----
    for b in range(BL):
        # E rows [a=src4+dst4, t] via two 512-col halves (z-pool slots)
        for half in range(2):
            e8 = ps_z.tile([128, 512], F32, tag="z")
            nc.tensor.matmul(e8[0:36, :], lhsT=WAcat_sb,
                             rhs=xT_sb[:, b, half * 512:(half + 1) * 512],
                             start=True, stop=True)
            sl = slice(half * 512, (half + 1) * 512)
            if b == 0:
                nc.vector.tensor_copy(srow_sb[0:4, b, sl], e8[0:4, :])
            else:
                nc.scalar.activation(srow_sb[0:4, b, sl], e8[0:4, :], AF.Copy,
                                     bias=0.0, scale=1.0)
        # E cols [t, a] per 128-chunk; exp into V / w columns
        ecol_slot = ps_z.tile([128, 512], F32, tag="z", name=f"ecol_{b}")
        ecol = ecol_slot[:, 0:NTC * 36]
        for tcc in range(NTC):
            nc.tensor.matmul(ecol[:, tcc * 36:(tcc + 1) * 36],
                             lhsT=xT_sb[:, b, tcc * 128:(tcc + 1) * 128],
                             rhs=WAcat_sb, start=True, stop=True)
        dstv = ecol.rearrange("p (t a) -> p t a", t=NTC)[:, :, 32:36]
        nc.scalar.activation(Vcol_sb[:, b], dstv, AF.Exp, bias=0.0, scale=1.0)
        nc.scalar.activation(wcol_sb[:, b], dstv, AF.Exp, bias=0.0, scale=0.2)
        # broadcast e_src row h across partitions via one-hot K=4 matmul,
        # then exp(0.8 x) straight from PSUM into the bf16 U8bc tile
        for h in range(H):
            for half in range(2):
                sl = slice(half * 512, (half + 1) * 512)
                ebc = ps_z.tile([128, 512], F32, tag="z", name=f"ebc_{b}_{h}_{half}")
                nc.tensor.matmul(ebc, lhsT=onehot_sb[:, h * 128:(h + 1) * 128],
                                 rhs=srow_sb[0:4, b, sl], start=True, stop=True)
                nc.scalar.activation(U8bc[:, b, h, sl], ebc, AF.Exp,
                                     bias=0.0, scale=0.8)
        # haug[t, h*64+d] = h in bf16 for the alpha@h contraction
        for tcc in range(NTC):
            hp = ps_p1.tile([128, HD], F32, tag="haug")
            nc.tensor.matmul(hp, lhsT=xT_sb[:, b, tcc * 128:(tcc + 1) * 128],
                             rhs=W_sb, start=True, stop=True)
            nc.scalar.activation(haug_sb[:, b, tcc, :], hp, AF.Copy,
                                 bias=0.0, scale=1.0)

    # ---- phase 2: scores + alpha @ h ----
    for b in range(BL):
        # 4 oacc banks hold the 32 (ic,h) 64-col chains; rs holds row-sums
        obank = [ps_acc.tile([128, 512], F32, tag=f"oacc{k}", name=f"oacc{k}_{b}")
                 for k in range(4)]
        rs = ps_acc.tile([128, 32], F32, tag="rs")
        qts = {}
        if b == 0:
            # h-major warmup: fill the first 4 groups' qt tiles per-head so
            # the DVE queue never blocks on a not-yet-exp'd U8bc head
            for jc in range(4):
                qts[jc] = qt_pool.tile([128, H, N], BF16, tag="qt",
                                       name=f"qtw_{b}_{jc}")
            for h in range(H):
                for jc in range(4):
                    nc.vector.tensor_scalar(
                        out=qts[jc][:, h, :], in0=U8bc[:, b, h, :],
                        scalar1=Vcol_sb[:, b, jc, h:h + 1],
                        scalar2=wcol_sb[:, b, jc, h:h + 1],
                        op0=ALU.mult, op1=ALU.max)
        for jc in range(NTC):
            qwm = qwm_pool.tile([128, H, N], BF16, tag="qwm")
            if (b, jc) in POOL_GROUPS:
                qt = qt_pool.tile([128, H, N], BF16, tag="qt")
                for h in range(H):
                    # u8_i * V_j via ACT copy with column scale
                    nc.scalar.activation(qt[:, h, :], U8bc[:, b, h, :], AF.Copy,
                                         bias=0.0,
                                         scale=Vcol_sb[:, b, jc, h:h + 1])
                    # (u8*V max w) * m on GPSIMD, all SBUF
                    nc.gpsimd.scalar_tensor_tensor(
                        out=qwm[:, h, :], in0=qt[:, h, :],
                        scalar=wcol_sb[:, b, jc, h:h + 1],
                        in1=maskT_sb[:, jc, :],
                        op0=ALU.max, op1=ALU.mult)
            else:
                if jc in qts:
                    qt = qts.pop(jc)
                else:
                    qt = qt_pool.tile([128, H, N], BF16, tag="qt")
                    for h in range(H):
                        nc.vector.tensor_scalar(
                            out=qt[:, h, :], in0=U8bc[:, b, h, :],
                            scalar1=Vcol_sb[:, b, jc, h:h + 1],
                            scalar2=wcol_sb[:, b, jc, h:h + 1],
                            op0=ALU.mult, op1=ALU.max)
                nc.vector.tensor_tensor(
                    out=qwm, in0=qt,
                    in1=maskT_sb[:, jc, :].unsqueeze(1).broadcast_to((128, H, N)),
                    op=ALU.mult)
            # start=True zeroes a whole 2KB psum bank: only the first chain in
            # each bank starts the group, only the last one stops it. In the
            # final group the row-sum matmuls go first so the reciprocal can
            # start while the data chains finish.
            passes = ([("rs",), ("data",)] if jc == NTC - 1
                      else [("data", "rs")])
            for kinds in passes:
                for h in range(H):
                    for ic in range(NTC):
                        c = ic * 4 + h
                        lhsT = qwm[:, h, ic * 128:(ic + 1) * 128]
                        if "data" in kinds:
                            nc.tensor.matmul(
                                obank[c // 8][:, (c % 8) * 64:(c % 8 + 1) * 64],
                                lhsT=lhsT,
                                rhs=haug_sb[:, b, jc, h * 64:(h + 1) * 64],
                                start=(jc == 0 and c % 8 == 0),
                                stop=(jc == NTC - 1 and c % 8 == 7))
                        if "rs" in kinds:
                            nc.tensor.matmul(rs[:, c:c + 1], lhsT=lhsT,
                                             rhs=ones_col,
                                             start=(jc == 0 and c == 0),
                                             stop=(jc == NTC - 1 and c == 31))
        if taps and b == 0:
            nc.sync.dma_start(out=taps["u8bc"], in_=U8bc[:, 0, 0, :])

            nc.sync.dma_start(out=taps["vcol"], in_=Vcol_sb[:, 0])
            nc.sync.dma_start(out=taps["wcol"], in_=wcol_sb[:, 0])
            rs_tap = osb_pool.tile([128, 32], F32, tag="rstap")
            nc.vector.tensor_copy(rs_tap, rs)
            nc.sync.dma_start(out=taps["rs"], in_=rs_tap)
            nc.sync.dma_start(out=taps["haug"], in_=haug_sb[:, 0])
        rcl = rcl_pool.tile([128, 32], F32, tag="rcl")
        nc.vector.reciprocal(rcl, rs)
        osb = osb_pool.tile([128, NTC, HD], BF16, tag="osb")
        for ic in range(NTC):
            oslice = obank[ic // 2][:, (ic % 2) * 256:(ic % 2 + 1) * 256]
            if b == 0:  # b0 norm on ACT (overlaps b1 scores); b1 on idle-tail DVE
                for h in range(H):
                    nc.scalar.activation(
                        osb[:, ic, h * 64:(h + 1) * 64],
                        oslice[:, h * 64:(h + 1) * 64], AF.Copy,
                        bias=0.0, scale=rcl[:, ic * 4 + h:ic * 4 + h + 1])
            else:
                nc.vector.tensor_tensor(
                    out=osb[:, ic, :].rearrange("p (h d) -> p h d", h=H),
                    in0=oslice.rearrange("p (h d) -> p h d", h=H),
                    in1=rcl[:, ic * 4:(ic + 1) * 4].unsqueeze(2)
                        .broadcast_to((128, 4, D)), op=ALU.mult)
        nc.sync.dma_start(
            out=out_d[b].rearrange("(ic p) d -> p ic d", p=128), in_=osb)


def prep_inputs(x, adj, W, a_src, a_dst):
    """Host-side prep: shard x over cores, build combined weight layouts."""
    x = np.asarray(x, np.float32)
    adj = np.asarray(adj)
    W = np.asarray(W, np.float32)
    a_src = np.asarray(a_src, np.float32)
    a_dst = np.asarray(a_dst, np.float32)

    maskT = np.ascontiguousarray(adj.T).astype(NPBF)
    Acat = np.zeros((HD, 36), np.float32)
    for h in range(H):
        Acat[h * D:(h + 1) * D, h] = a_src[h]
        Acat[h * D:(h + 1) * D, 32 + h] = a_dst[h]
    WAcat = np.ascontiguousarray(W @ Acat)  # (IN_DIM, 36): src at 0-3, dst at 32-35

    onehot = np.zeros((4, 4 * 128), np.float32)
    for h in range(H):
        onehot[h, h * 128:(h + 1) * 128] = 1.0

    in_maps = []
    for c in range(NCORES):
        xT = np.ascontiguousarray(x[c * BL:(c + 1) * BL].transpose(0, 2, 1))
        in_maps.append({"xT": xT, "W": W, "WAcat": WAcat, "maskT": maskT,
                        "onehot": onehot})
    return in_maps


_PROGRAM_CACHE = {}


def _get_program():
    if "nc" not in _PROGRAM_CACHE:
        _PROGRAM_CACHE["nc"] = build_gat_program()
    return _PROGRAM_CACHE["nc"]


def run_on_hw(inputs, trace=False):
    from concourse.bass_utils import run_bass_kernel_spmd
    nc = _get_program()
    in_maps = prep_inputs(**inputs)
    res = run_bass_kernel_spmd(nc, in_maps, list(range(NCORES)), trace=trace)
    out = np.concatenate(
        [np.asarray(res.results[c]["out"]).astype(np.float32)
         for c in range(NCORES)], axis=0)
    return out, res


def kernel(**inputs) -> np.ndarray:
    out, _ = run_on_hw(inputs, trace=False)
    return out


# revision 36
# speedup vs baseline: 1.1250x; 1.0057x over previous
"""GAT layer kernel for Trainium2 (Bass/Tile), 8-core data-parallel over batch.

Reference (B=16, N=1024, IN_DIM=128, H=4, D=64):
    h = (x @ W).reshape(B,N,H,D)
    e_src/e_dst = einsum('bnhd,hd->bnh', h, a_src/a_dst)
    e[b,i,j,h] = leakyrelu(e_src[b,i,h] + e_dst[b,j,h], 0.2)
    alpha = softmax_j(where(adj[i,j], e, -inf));  out = alpha @ h

Kernel strategy (per core, 2 batches):
  Softmax shift-invariance: with y = s_i + d_j, lrelu(y) = 0.2 s_i + 0.2 d_j
  + 0.8 relu(y); the 0.2 s_i term is constant over j and cancels. So the
  (unnormalized) score reduces to
      PT[j,i] = max(u8_i * V_j, w_j) * m[j,i]
  with u8 = exp(0.8 e_src), V = exp(e_dst), w = exp(0.2 e_dst): one fused DVE
  tensor_scalar (mult, max) per (b,jc,h) against a partition-broadcast u8
  tile, plus one mask multiply shared across the 4 heads. A second path runs
  entirely on PE (rank-1 outer product u8 x V) + GPSIMD (fused (z max w)*m),
  soaking otherwise-idle engines. Row-sums ride separate 1-column matmuls;
  normalization is a batched reciprocal + broadcast multiply.
  All heavy matmuls use bf16 or fp32r (1 PE cycle/row vs 4 for fp32).
"""

import os
import sys
from contextlib import ExitStack

import numpy as np
import ml_dtypes

for _p in ("/opt/trn_rl_repo", "/root/.axon_site/_ro/trn_rl_repo"):
    if os.path.isdir(_p) and _p not in sys.path:
        sys.path.insert(0, _p)

import concourse.bass as bass
import concourse.mybir as mybir
import concourse.tile as tile

F32 = mybir.dt.float32
F32R = mybir.dt.float32r
BF16 = mybir.dt.bfloat16
AF = mybir.ActivationFunctionType
ALU = mybir.AluOpType
NPBF = ml_dtypes.bfloat16

B, N, IN_DIM, H, D = 16, 1024, 128, 4, 64
HD = H * D            # 256
NCORES = 8
BL = B // NCORES      # 2 batches per core
NTC = N // 128        # 8 chunks of 128

# (b, jc) score groups handled by the PE-outer-product + GPSIMD path;
# the rest go through the DVE tensor_scalar path.
POOL_GROUPS = set()
DEBUG_TAPS = False


def _split_excess_waits(nc, max_waits=1):
    """Walrus codegen rejects compute instructions carrying more than one
    sync wait. Move the extras onto engine-matched NoOps inserted
    immediately before the instruction."""
    def _steal_nop(engine):
        engine.nop()
        for fn in nc.m.functions:
            for blk in fn.blocks:
                il = blk.instructions
                if il and type(il[-1]).__name__ == "InstNoOp":
                    nop = il[-1]
                    blk.instructions = il[:-1]
                    return nop
        raise RuntimeError("could not locate appended nop")

    for fn in nc.m.functions:
        for blk in fn.blocks:
            il = list(blk.instructions)
            out = []
            changed = False
            for inst in il:
                si = inst.sync_info
                if (type(inst).__name__ != "InstNoOp" and si is not None
                        and len(si.on_wait) > max_waits):
                    waits = list(si.on_wait)
                    for w in waits[max_waits:]:
                        nop = _steal_nop(nc.engines[inst.engine])
                        nop.sync_info = mybir.SyncInfo(on_wait=[w], on_update=[])
                        out.append(nop)
                    inst.sync_info = mybir.SyncInfo(
                        on_wait=waits[:max_waits], on_update=list(si.on_update))
                    changed = True
                out.append(inst)
            if changed:
                blk.instructions = out


def build_gat_program():
    nc = bass.Bass("TRN2", target_bir_lowering=False, debug=False)
    xT_d = nc.dram_tensor("xT", (BL, IN_DIM, N), F32R, kind="ExternalInput").ap()
    W_d = nc.dram_tensor("W", (IN_DIM, HD), F32R, kind="ExternalInput").ap()
    WAcat_d = nc.dram_tensor("WAcat", (IN_DIM, 36), F32R, kind="ExternalInput").ap()
    maskT_d = nc.dram_tensor("maskT", (N, N), BF16, kind="ExternalInput").ap()
    onehot_d = nc.dram_tensor("onehot", (4, 4 * 128), F32R, kind="ExternalInput").ap()
    out_d = nc.dram_tensor("out", (BL, N, HD), BF16, kind="ExternalOutput").ap()
    taps = {}
    if DEBUG_TAPS:
        taps["u8bc"] = nc.dram_tensor("t_u8bc", (128, N), BF16, kind="ExternalOutput").ap()

        taps["vcol"] = nc.dram_tensor("t_vcol", (128, NTC, H), F32, kind="ExternalOutput").ap()
        taps["wcol"] = nc.dram_tensor("t_wcol", (128, NTC, H), F32, kind="ExternalOutput").ap()
        taps["qwm"] = nc.dram_tensor("t_qwm", (128, H, N), BF16, kind="ExternalOutput").ap()
        taps["rs"] = nc.dram_tensor("t_rs", (128, 32), F32, kind="ExternalOutput").ap()
        taps["haug"] = nc.dram_tensor("t_haug", (128, NTC, HD), BF16, kind="ExternalOutput").ap()

    with tile.TileContext(nc) as tc:
        with ExitStack() as ctx:
            _gat_body(ctx, tc, out_d, xT_d, W_d, WAcat_d, maskT_d, onehot_d,
                      taps)
    _split_excess_waits(nc)
    return nc


def _gat_body(ctx, tc, out_d, xT_d, W_d, WAcat_d, maskT_d, onehot_d, taps=None):
    nc = tc.nc

    consts = ctx.enter_context(tc.tile_pool(name="consts", bufs=1))
    persist = ctx.enter_context(tc.tile_pool(name="persist", bufs=1))
    qt_pool = ctx.enter_context(tc.tile_pool(name="qt", bufs=4))
    qwm_pool = ctx.enter_context(tc.tile_pool(name="qwm", bufs=6))
    osb_pool = ctx.enter_context(tc.tile_pool(name="osb", bufs=3))
    rcl_pool = ctx.enter_context(tc.tile_pool(name="rcl", bufs=3))
    ps_z = ctx.enter_context(tc.tile_pool(name="ps_z", bufs=2, space="PSUM"))
    ps_p1 = ctx.enter_context(tc.tile_pool(name="ps_p1", bufs=1, space="PSUM"))
    ps_acc = ctx.enter_context(tc.tile_pool(name="ps_acc", bufs=1, space="PSUM"))

    # ---- constants / inputs resident in SBUF ----
    # xT b0 first: its descriptor-gen overlaps the tiny weight transfers
    xT_sb = consts.tile([128, BL, N], F32R)
    nc.sync.dma_start(out=xT_sb[:, 0, :], in_=xT_d[0])
    WAcat_sb = consts.tile([128, 36], F32R)
    nc.sync.dma_start(out=WAcat_sb, in_=WAcat_d)
    onehot_sb = consts.tile([4, 4 * 128], F32R)
    nc.sync.dma_start(out=onehot_sb, in_=onehot_d)
    nc.sync.dma_start(out=xT_sb[:, 1, :], in_=xT_d[1])
    W_sb = consts.tile([128, HD], F32R)
    nc.sync.dma_start(out=W_sb, in_=W_d)
    ones_col = consts.tile([128, 1], BF16)
    nc.vector.memset(ones_col, 1.0)
    maskT_sb = consts.tile([128, NTC, N], BF16)
    nc.sync.dma_start(
        out=maskT_sb,
        in_=maskT_d.rearrange("(jc p) i -> p jc i", p=128))

    # ---- persistent per-batch intermediates ----
    haug_sb = persist.tile([128, BL, NTC, HD], BF16)   # [j-in-chunk, b, jc, h*64+d]
    srow_sb = persist.tile([4, BL, N], F32R)           # raw e_src rows
    Vcol_sb = persist.tile([128, BL, NTC, H], F32)     # exp(e_dst) cols
    wcol_sb = persist.tile([128, BL, NTC, H], F32)     # exp(0.2 e_dst) cols
    U8bc = persist.tile([128, BL, H, N], BF16)         # u8 broadcast over parts

    # ---- phase 1: E = x @ WAcat (rows + cols), haug = x @ W ----
    for b in range(BL):
        # E rows [a=src4+dst4, t] via two 512-col halves (z-pool slots)
        for half in range(2):
            e8 = ps_z.tile([128, 512], F32, tag="z")
            nc.tensor.matmul(e8[0:36, :], lhsT=WAcat_sb,
                             rhs=xT_sb[:, b, half * 512:(half + 1) * 512],
                             start=True, stop=True)
            sl = slice(half * 512, (half + 1) * 512)
            if b == 0:
                nc.vector.tensor_copy(srow_sb[0:4, b, sl], e8[0:4, :])
            else:
                nc.scalar.activation(srow_sb[0:4, b, sl], e8[0:4, :], AF.Copy,
                                     bias=0.0, scale=1.0)
        # E cols [t, a] per 128-chunk; exp into V / w columns
        ecol_slot = ps_z.tile([128, 512], F32, tag="z", name=f"ecol_{b}")
        ecol = ecol_slot[:, 0:NTC * 36]
        for tcc in range(NTC):
            nc.tensor.matmul(ecol[:, tcc * 36:(tcc + 1) * 36],
                             lhsT=xT_sb[:, b, tcc * 128:(tcc + 1) * 128],
                             rhs=WAcat_sb, start=True, stop=True)
        dstv = ecol.rearrange("p (t a) -> p t a", t=NTC)[:, :, 32:36]
        nc.scalar.activation(Vcol_sb[:, b], dstv, AF.Exp, bias=0.0, scale=1.0)
        nc.scalar.activation(wcol_sb[:, b], dstv, AF.Exp, bias=0.0, scale=0.2)
        # broadcast e_src row h across partitions via one-hot K=4 matmul,
        # then exp(0.8 x) straight from PSUM into the bf16 U8bc tile
        for h in range(H):
            for half in range(2):
                sl = slice(half * 512, (half + 1) * 512)
                ebc = ps_z.tile([128, 512], F32, tag="z", name=f"ebc_{b}_{h}_{half}")
                nc.tensor.matmul(ebc, lhsT=onehot_sb[:, h * 128:(h + 1) * 128],
                                 rhs=srow_sb[0:4, b, sl], start=True, stop=True)
                nc.scalar.activation(U8bc[:, b, h, sl], ebc, AF.Exp,
                                     bias=0.0, scale=0.8)
        # haug[t, h*64+d] = h in bf16 for the alpha@h contraction
        for tcc in range(NTC):
            hp = ps_p1.tile([128, HD], F32, tag="haug")
            nc.tensor.matmul(hp, lhsT=xT_sb[:, b, tcc * 128:(tcc + 1) * 128],
                             rhs=W_sb, start=True, stop=True)
            nc.scalar.activation(haug_sb[:, b, tcc, :], hp, AF.Copy,
                                 bias=0.0, scale=1.0)

    # ---- phase 2: scores + alpha @ h ----
    for b in range(BL):
        # 4 oacc banks hold the 32 (ic,h) 64-col chains; rs holds row-sums
        obank = [ps_acc.tile([128, 512], F32, tag=f"oacc{k}", name=f"oacc{k}_{b}")
                 for k in range(4)]
        rs = ps_acc.tile([128, 32], F32, tag="rs")
        qts = {}
        if b == 0:
            # h-major warmup: fill the first 4 groups' qt tiles per-head so
            # the DVE queue never blocks on a not-yet-exp'd U8bc head
            for jc in range(4):
                qts[jc] = qt_pool.tile([128, H, N], BF16, tag="qt",
                                       name=f"qtw_{b}_{jc}")
            for h in range(H):
                for jc in range(4):
                    nc.vector.tensor_scalar(
                        out=qts[jc][:, h, :], in0=U8bc[:, b, h, :],
                        scalar1=Vcol_sb[:, b, jc, h:h + 1],
                        scalar2=wcol_sb[:, b, jc, h:h + 1],
                        op0=ALU.mult, op1=ALU.max)
        for jc in range(NTC):
            qwm = qwm_pool.tile([128, H, N], BF16, tag="qwm")
            if (b, jc) in POOL_GROUPS:
                qt = qt_pool.tile([128, H, N], BF16, tag="qt")
                for h in range(H):
                    # u8_i * V_j via ACT copy with column scale
                    nc.scalar.activation(qt[:, h, :], U8bc[:, b, h, :], AF.Copy,
                                         bias=0.0,
                                         scale=Vcol_sb[:, b, jc, h:h + 1])
                    # (u8*V max w) * m on GPSIMD, all SBUF
                    nc.gpsimd.scalar_tensor_tensor(
                        out=qwm[:, h, :], in0=qt[:, h, :],
                        scalar=wcol_sb[:, b, jc, h:h + 1],
                        in1=maskT_sb[:, jc, :],
                        op0=ALU.max, op1=ALU.mult)
            else:
                if jc in qts:
                    qt = qts.pop(jc)
                else:
                    qt = qt_pool.tile([128, H, N], BF16, tag="qt")
                    for h in range(H):
                        nc.vector.tensor_scalar(
                            out=qt[:, h, :], in0=U8bc[:, b, h, :],
                            scalar1=Vcol_sb[:, b, jc, h:h + 1],
                            scalar2=wcol_sb[:, b, jc, h:h + 1],
                            op0=ALU.mult, op1=ALU.max)
                nc.vector.tensor_tensor(
                    out=qwm, in0=qt,
                    in1=maskT_sb[:, jc, :].unsqueeze(1).broadcast_to((128, H, N)),
                    op=ALU.mult)
            # start=True zeroes a whole 2KB psum bank: only the first chain in
            # each bank starts the group, only the last one stops it. In the
            # final group the row-sum matmuls go first so the reciprocal can
            # start while the data chains finish.
            passes = ([("rs",), ("data",)] if jc == NTC - 1
                      else [("data", "rs")])
            for kinds in passes:
                for h in range(H):
                    for ic in range(NTC):
                        c = ic * 4 + h
                        lhsT = qwm[:, h, ic * 128:(ic + 1) * 128]
                        if "data" in kinds:
                            nc.tensor.matmul(
                                obank[c // 8][:, (c % 8) * 64:(c % 8 + 1) * 64],
                                lhsT=lhsT,
                                rhs=haug_sb[:, b, jc, h * 64:(h + 1) * 64],
                                start=(jc == 0 and c % 8 == 0),
                                stop=(jc == NTC - 1 and c % 8 == 7))
                        if "rs" in kinds:
                            nc.tensor.matmul(rs[:, c:c + 1], lhsT=lhsT,
                                             rhs=ones_col,
                                             start=(jc == 0 and c == 0),
                                             stop=(jc == NTC - 1 and c == 31))
        if taps and b == 0:
            nc.sync.dma_start(out=taps["u8bc"], in_=U8bc[:, 0, 0, :])

            nc.sync.dma_start(out=taps["vcol"], in_=Vcol_sb[:, 0])
            nc.sync.dma_start(out=taps["wcol"], in_=wcol_sb[:, 0])
            rs_tap = osb_pool.tile([128, 32], F32, tag="rstap")
            nc.vector.tensor_copy(rs_tap, rs)
            nc.sync.dma_start(out=taps["rs"], in_=rs_tap)
            nc.sync.dma_start(out=taps["haug"], in_=haug_sb[:, 0])
        rcl = rcl_pool.tile([128, 32], F32, tag="rcl")
        nc.vector.reciprocal(rcl, rs)
        osb = osb_pool.tile([128, NTC, HD], BF16, tag="osb")
        half_out = NTC // 2
        for ic in range(NTC):
            oslice = obank[ic // 2][:, (ic % 2) * 256:(ic % 2 + 1) * 256]
            if b == 0:  # b0 norm on ACT (overlaps b1 scores); b1 on idle-tail DVE
                for h in range(H):
                    nc.scalar.activation(
                        osb[:, ic, h * 64:(h + 1) * 64],
                        oslice[:, h * 64:(h + 1) * 64], AF.Copy,
                        bias=0.0, scale=rcl[:, ic * 4 + h:ic * 4 + h + 1])
            else:
                nc.vector.tensor_tensor(
                    out=osb[:, ic, :].rearrange("p (h d) -> p h d", h=H),
                    in0=oslice.rearrange("p (h d) -> p h d", h=H),
                    in1=rcl[:, ic * 4:(ic + 1) * 4].unsqueeze(2)
                        .broadcast_to((128, 4, D)), op=ALU.mult)
            if ic == half_out - 1:
                nc.sync.dma_start(
                    out=out_d[b, 0:half_out * 128].rearrange(
                        "(ic p) d -> p ic d", p=128),
                    in_=osb[:, 0:half_out, :])
        nc.sync.dma_start(
            out=out_d[b, half_out * 128:].rearrange("(ic p) d -> p ic d", p=128),
            in_=osb[:, half_out:, :])


def prep_inputs(x, adj, W, a_src, a_dst):
    """Host-side prep: shard x over cores, build combined weight layouts."""
    x = np.asarray(x, np.float32)
    adj = np.asarray(adj)
    W = np.asarray(W, np.float32)
    a_src = np.asarray(a_src, np.float32)
    a_dst = np.asarray(a_dst, np.float32)

    maskT = np.ascontiguousarray(adj.T).astype(NPBF)
    Acat = np.zeros((HD, 36), np.float32)
    for h in range(H):
        Acat[h * D:(h + 1) * D, h] = a_src[h]
        Acat[h * D:(h + 1) * D, 32 + h] = a_dst[h]
    WAcat = np.ascontiguousarray(W @ Acat)  # (IN_DIM, 36): src at 0-3, dst at 32-35

    onehot = np.zeros((4, 4 * 128), np.float32)
    for h in range(H):
        onehot[h, h * 128:(h + 1) * 128] = 1.0

    in_maps = []
    for c in range(NCORES):
        xT = np.ascontiguousarray(x[c * BL:(c + 1) * BL].transpose(0, 2, 1))
        in_maps.append({"xT": xT, "W": W, "WAcat": WAcat, "maskT": maskT,
                        "onehot": onehot})
    return in_maps


_PROGRAM_CACHE = {}


def _get_program():
    if "nc" not in _PROGRAM_CACHE:
        _PROGRAM_CACHE["nc"] = build_gat_program()
    return _PROGRAM_CACHE["nc"]


def run_on_hw(inputs, trace=False):
    from concourse.bass_utils import run_bass_kernel_spmd
    nc = _get_program()
    in_maps = prep_inputs(**inputs)
    res = run_bass_kernel_spmd(nc, in_maps, list(range(NCORES)), trace=trace)
    out = np.concatenate(
        [np.asarray(res.results[c]["out"]).astype(np.float32)
         for c in range(NCORES)], axis=0)
    return out, res


def kernel(**inputs) -> np.ndarray:
    out, _ = run_on_hw(inputs, trace=False)
    return out


# revision 37
# speedup vs baseline: 1.1258x; 1.0007x over previous
"""GAT layer kernel for Trainium2 (Bass/Tile), 8-core data-parallel over batch.

Reference (B=16, N=1024, IN_DIM=128, H=4, D=64):
    h = (x @ W).reshape(B,N,H,D)
    e_src/e_dst = einsum('bnhd,hd->bnh', h, a_src/a_dst)
    e[b,i,j,h] = leakyrelu(e_src[b,i,h] + e_dst[b,j,h], 0.2)
    alpha = softmax_j(where(adj[i,j], e, -inf));  out = alpha @ h

Kernel strategy (per core, 2 batches):
  Softmax shift-invariance: with y = s_i + d_j, lrelu(y) = 0.2 s_i + 0.2 d_j
  + 0.8 relu(y); the 0.2 s_i term is constant over j and cancels. So the
  (unnormalized) score reduces to
      PT[j,i] = max(u8_i * V_j, w_j) * m[j,i]
  with u8 = exp(0.8 e_src), V = exp(e_dst), w = exp(0.2 e_dst): one fused DVE
  tensor_scalar (mult, max) per (b,jc,h) against a partition-broadcast u8
  tile, plus one mask multiply shared across the 4 heads. A second path runs
  entirely on PE (rank-1 outer product u8 x V) + GPSIMD (fused (z max w)*m),
  soaking otherwise-idle engines. Row-sums ride separate 1-column matmuls;
  normalization is a batched reciprocal + broadcast multiply.
  All heavy matmuls use bf16 or fp32r (1 PE cycle/row vs 4 for fp32).
"""

import os
import sys
from contextlib import ExitStack

import numpy as np
import ml_dtypes

for _p in ("/opt/trn_rl_repo", "/root/.axon_site/_ro/trn_rl_repo"):
    if os.path.isdir(_p) and _p not in sys.path:
        sys.path.insert(0, _p)

import concourse.bass as bass
import concourse.mybir as mybir
import concourse.tile as tile

F32 = mybir.dt.float32
F32R = mybir.dt.float32r
BF16 = mybir.dt.bfloat16
AF = mybir.ActivationFunctionType
ALU = mybir.AluOpType
NPBF = ml_dtypes.bfloat16

B, N, IN_DIM, H, D = 16, 1024, 128, 4, 64
HD = H * D            # 256
NCORES = 8
BL = B // NCORES      # 2 batches per core
NTC = N // 128        # 8 chunks of 128

# (b, jc) score groups handled by the PE-outer-product + GPSIMD path;
# the rest go through the DVE tensor_scalar path.
POOL_GROUPS = set()
DEBUG_TAPS = False


def _split_excess_waits(nc, max_waits=1):
    """Walrus codegen rejects compute instructions carrying more than one
    sync wait. Move the extras onto engine-matched NoOps inserted
    immediately before the instruction."""
    def _steal_nop(engine):
        engine.nop()
        for fn in nc.m.functions:
            for blk in fn.blocks:
                il = blk.instructions
                if il and type(il[-1]).__name__ == "InstNoOp":
                    nop = il[-1]
                    blk.instructions = il[:-1]
                    return nop
        raise RuntimeError("could not locate appended nop")

    for fn in nc.m.functions:
        for blk in fn.blocks:
            il = list(blk.instructions)
            out = []
            changed = False
            for inst in il:
                si = inst.sync_info
                if (type(inst).__name__ != "InstNoOp" and si is not None
                        and len(si.on_wait) > max_waits):
                    waits = list(si.on_wait)
                    for w in waits[max_waits:]:
                        nop = _steal_nop(nc.engines[inst.engine])
                        nop.sync_info = mybir.SyncInfo(on_wait=[w], on_update=[])
                        out.append(nop)
                    inst.sync_info = mybir.SyncInfo(
                        on_wait=waits[:max_waits], on_update=list(si.on_update))
                    changed = True
                out.append(inst)
            if changed:
                blk.instructions = out


def build_gat_program():
    nc = bass.Bass("TRN2", target_bir_lowering=False, debug=False)
    xT_d = nc.dram_tensor("xT", (BL, IN_DIM, N), F32R, kind="ExternalInput").ap()
    W_d = nc.dram_tensor("W", (IN_DIM, HD), F32R, kind="ExternalInput").ap()
    WAcat_d = nc.dram_tensor("WAcat", (IN_DIM, 36), F32R, kind="ExternalInput").ap()
    maskT_d = nc.dram_tensor("maskT", (N, N), BF16, kind="ExternalInput").ap()
    onehot_d = nc.dram_tensor("onehot", (4, 4 * 128), F32R, kind="ExternalInput").ap()
    out_d = nc.dram_tensor("out", (BL, N, HD), BF16, kind="ExternalOutput").ap()
    taps = {}
    if DEBUG_TAPS:
        taps["u8bc"] = nc.dram_tensor("t_u8bc", (128, N), BF16, kind="ExternalOutput").ap()

        taps["vcol"] = nc.dram_tensor("t_vcol", (128, NTC, H), F32, kind="ExternalOutput").ap()
        taps["wcol"] = nc.dram_tensor("t_wcol", (128, NTC, H), F32, kind="ExternalOutput").ap()
        taps["qwm"] = nc.dram_tensor("t_qwm", (128, H, N), BF16, kind="ExternalOutput").ap()
        taps["rs"] = nc.dram_tensor("t_rs", (128, 32), F32, kind="ExternalOutput").ap()
        taps["haug"] = nc.dram_tensor("t_haug", (128, NTC, HD), BF16, kind="ExternalOutput").ap()

    with tile.TileContext(nc) as tc:
        with ExitStack() as ctx:
            _gat_body(ctx, tc, out_d, xT_d, W_d, WAcat_d, maskT_d, onehot_d,
                      taps)
    _split_excess_waits(nc)
    return nc


def _gat_body(ctx, tc, out_d, xT_d, W_d, WAcat_d, maskT_d, onehot_d, taps=None):
    nc = tc.nc

    consts = ctx.enter_context(tc.tile_pool(name="consts", bufs=1))
    persist = ctx.enter_context(tc.tile_pool(name="persist", bufs=1))
    qt_pool = ctx.enter_context(tc.tile_pool(name="qt", bufs=4))
    qwm_pool = ctx.enter_context(tc.tile_pool(name="qwm", bufs=6))
    osb_pool = ctx.enter_context(tc.tile_pool(name="osb", bufs=3))
    rcl_pool = ctx.enter_context(tc.tile_pool(name="rcl", bufs=3))
    ps_z = ctx.enter_context(tc.tile_pool(name="ps_z", bufs=2, space="PSUM"))
    ps_p1 = ctx.enter_context(tc.tile_pool(name="ps_p1", bufs=1, space="PSUM"))
    ps_acc = ctx.enter_context(tc.tile_pool(name="ps_acc", bufs=1, space="PSUM"))

    # ---- constants / inputs resident in SBUF ----
    # xT b0 first: its descriptor-gen overlaps the tiny weight transfers
    xT_sb = consts.tile([128, BL, N], F32R)
    nc.sync.dma_start(out=xT_sb[:, 0, 0:512], in_=xT_d[0][:, 0:512])
    nc.sync.dma_start(out=xT_sb[:, 0, 512:], in_=xT_d[0][:, 512:])
    WAcat_sb = consts.tile([128, 36], F32R)
    nc.sync.dma_start(out=WAcat_sb, in_=WAcat_d)
    onehot_sb = consts.tile([4, 4 * 128], F32R)
    nc.sync.dma_start(out=onehot_sb, in_=onehot_d)
    nc.sync.dma_start(out=xT_sb[:, 1, :], in_=xT_d[1])
    W_sb = consts.tile([128, HD], F32R)
    nc.sync.dma_start(out=W_sb, in_=W_d)
    ones_col = consts.tile([128, 1], BF16)
    nc.vector.memset(ones_col, 1.0)
    maskT_sb = consts.tile([128, NTC, N], BF16)
    nc.sync.dma_start(
        out=maskT_sb,
        in_=maskT_d.rearrange("(jc p) i -> p jc i", p=128))

    # ---- persistent per-batch intermediates ----
    haug_sb = persist.tile([128, BL, NTC, HD], BF16)   # [j-in-chunk, b, jc, h*64+d]
    srow_sb = persist.tile([4, BL, N], F32R)           # raw e_src rows
    Vcol_sb = persist.tile([128, BL, NTC, H], F32)     # exp(e_dst) cols
    wcol_sb = persist.tile([128, BL, NTC, H], F32)     # exp(0.2 e_dst) cols
    U8bc = persist.tile([128, BL, H, N], BF16)         # u8 broadcast over parts

    # ---- phase 1: E = x @ WAcat (rows + cols), haug = x @ W ----
    for b in range(BL):
        # E rows [a=src4+dst4, t] via two 512-col halves (z-pool slots)
        for half in range(2):
            e8 = ps_z.tile([128, 512], F32, tag="z")
            nc.tensor.matmul(e8[0:36, :], lhsT=WAcat_sb,
                             rhs=xT_sb[:, b, half * 512:(half + 1) * 512],
                             start=True, stop=True)
            sl = slice(half * 512, (half + 1) * 512)
            if b == 0:
                nc.vector.tensor_copy(srow_sb[0:4, b, sl], e8[0:4, :])
            else:
                nc.scalar.activation(srow_sb[0:4, b, sl], e8[0:4, :], AF.Copy,
                                     bias=0.0, scale=1.0)
        # E cols [t, a] per 128-chunk; exp into V / w columns
        ecol_slot = ps_z.tile([128, 512], F32, tag="z", name=f"ecol_{b}")
        ecol = ecol_slot[:, 0:NTC * 36]
        for tcc in range(NTC):
            nc.tensor.matmul(ecol[:, tcc * 36:(tcc + 1) * 36],
                             lhsT=xT_sb[:, b, tcc * 128:(tcc + 1) * 128],
                             rhs=WAcat_sb, start=True, stop=True)
        dstv = ecol.rearrange("p (t a) -> p t a", t=NTC)[:, :, 32:36]
        nc.scalar.activation(Vcol_sb[:, b], dstv, AF.Exp, bias=0.0, scale=1.0)
        nc.scalar.activation(wcol_sb[:, b], dstv, AF.Exp, bias=0.0, scale=0.2)
        # broadcast e_src row h across partitions via one-hot K=4 matmul,
        # then exp(0.8 x) straight from PSUM into the bf16 U8bc tile
        for h in range(H):
            for half in range(2):
                sl = slice(half * 512, (half + 1) * 512)
                ebc = ps_z.tile([128, 512], F32, tag="z", name=f"ebc_{b}_{h}_{half}")
                nc.tensor.matmul(ebc, lhsT=onehot_sb[:, h * 128:(h + 1) * 128],
                                 rhs=srow_sb[0:4, b, sl], start=True, stop=True)
                nc.scalar.activation(U8bc[:, b, h, sl], ebc, AF.Exp,
                                     bias=0.0, scale=0.8)
        # haug[t, h*64+d] = h in bf16 for the alpha@h contraction
        for tcc in range(NTC):
            hp = ps_p1.tile([128, HD], F32, tag="haug")
            nc.tensor.matmul(hp, lhsT=xT_sb[:, b, tcc * 128:(tcc + 1) * 128],
                             rhs=W_sb, start=True, stop=True)
            nc.scalar.activation(haug_sb[:, b, tcc, :], hp, AF.Copy,
                                 bias=0.0, scale=1.0)

    # ---- phase 2: scores + alpha @ h ----
    for b in range(BL):
        # 4 oacc banks hold the 32 (ic,h) 64-col chains; rs holds row-sums
        obank = [ps_acc.tile([128, 512], F32, tag=f"oacc{k}", name=f"oacc{k}_{b}")
                 for k in range(4)]
        rs = ps_acc.tile([128, 32], F32, tag="rs")
        qts = {}
        if b == 0:
            # h-major warmup: fill the first 4 groups' qt tiles per-head so
            # the DVE queue never blocks on a not-yet-exp'd U8bc head
            for jc in range(4):
                qts[jc] = qt_pool.tile([128, H, N], BF16, tag="qt",
                                       name=f"qtw_{b}_{jc}")
            for h in range(H):
                for jc in range(4):
                    nc.vector.tensor_scalar(
                        out=qts[jc][:, h, :], in0=U8bc[:, b, h, :],
                        scalar1=Vcol_sb[:, b, jc, h:h + 1],
                        scalar2=wcol_sb[:, b, jc, h:h + 1],
                        op0=ALU.mult, op1=ALU.max)
        for jc in range(NTC):
            qwm = qwm_pool.tile([128, H, N], BF16, tag="qwm")
            if (b, jc) in POOL_GROUPS:
                qt = qt_pool.tile([128, H, N], BF16, tag="qt")
                for h in range(H):
                    # u8_i * V_j via ACT copy with column scale
                    nc.scalar.activation(qt[:, h, :], U8bc[:, b, h, :], AF.Copy,
                                         bias=0.0,
                                         scale=Vcol_sb[:, b, jc, h:h + 1])
                    # (u8*V max w) * m on GPSIMD, all SBUF
                    nc.gpsimd.scalar_tensor_tensor(
                        out=qwm[:, h, :], in0=qt[:, h, :],
                        scalar=wcol_sb[:, b, jc, h:h + 1],
                        in1=maskT_sb[:, jc, :],
                        op0=ALU.max, op1=ALU.mult)
            else:
                if jc in qts:
                    qt = qts.pop(jc)
                else:
                    qt = qt_pool.tile([128, H, N], BF16, tag="qt")
                    for h in range(H):
                        nc.vector.tensor_scalar(
                            out=qt[:, h, :], in0=U8bc[:, b, h, :],
                            scalar1=Vcol_sb[:, b, jc, h:h + 1],
                            scalar2=wcol_sb[:, b, jc, h:h + 1],
                            op0=ALU.mult, op1=ALU.max)
                nc.vector.tensor_tensor(
                    out=qwm, in0=qt,
                    in1=maskT_sb[:, jc, :].unsqueeze(1).broadcast_to((128, H, N)),
                    op=ALU.mult)
            # start=True zeroes a whole 2KB psum bank: only the first chain in
            # each bank starts the group, only the last one stops it. In the
            # final group the row-sum matmuls go first so the reciprocal can
            # start while the data chains finish.
            passes = ([("rs",), ("data",)] if jc == NTC - 1
                      else [("data", "rs")])
            for kinds in passes:
                for h in range(H):
                    for ic in range(NTC):
                        c = ic * 4 + h
                        lhsT = qwm[:, h, ic * 128:(ic + 1) * 128]
                        if "data" in kinds:
                            nc.tensor.matmul(
                                obank[c // 8][:, (c % 8) * 64:(c % 8 + 1) * 64],
                                lhsT=lhsT,
                                rhs=haug_sb[:, b, jc, h * 64:(h + 1) * 64],
                                start=(jc == 0 and c % 8 == 0),
                                stop=(jc == NTC - 1 and c % 8 == 7))
                        if "rs" in kinds:
                            nc.tensor.matmul(rs[:, c:c + 1], lhsT=lhsT,
                                             rhs=ones_col,
                                             start=(jc == 0 and c == 0),
                                             stop=(jc == NTC - 1 and c == 31))
        if taps and b == 0:
            nc.sync.dma_start(out=taps["u8bc"], in_=U8bc[:, 0, 0, :])

            nc.sync.dma_start(out=taps["vcol"], in_=Vcol_sb[:, 0])
            nc.sync.dma_start(out=taps["wcol"], in_=wcol_sb[:, 0])
            rs_tap = osb_pool.tile([128, 32], F32, tag="rstap")
            nc.vector.tensor_copy(rs_tap, rs)
            nc.sync.dma_start(out=taps["rs"], in_=rs_tap)
            nc.sync.dma_start(out=taps["haug"], in_=haug_sb[:, 0])
        rcl = rcl_pool.tile([128, 32], F32, tag="rcl")
        nc.vector.reciprocal(rcl, rs)
        osb = osb_pool.tile([128, NTC, HD], BF16, tag="osb")
        half_out = NTC // 2
        for ic in range(NTC):
            oslice = obank[ic // 2][:, (ic % 2) * 256:(ic % 2 + 1) * 256]
            if b == 0:  # b0 norm on ACT (overlaps b1 scores); b1 on idle-tail DVE
                for h in range(H):
                    nc.scalar.activation(
                        osb[:, ic, h * 64:(h + 1) * 64],
                        oslice[:, h * 64:(h + 1) * 64], AF.Copy,
                        bias=0.0, scale=rcl[:, ic * 4 + h:ic * 4 + h + 1])
            else:
                nc.vector.tensor_tensor(
                    out=osb[:, ic, :].rearrange("p (h d) -> p h d", h=H),
                    in0=oslice.rearrange("p (h d) -> p h d", h=H),
                    in1=rcl[:, ic * 4:(ic + 1) * 4].unsqueeze(2)
                        .broadcast_to((128, 4, D)), op=ALU.mult)
            if ic == half_out - 1:
                nc.sync.dma_start(
                    out=out_d[b, 0:half_out * 128].rearrange(
                        "(ic p) d -> p ic d", p=128),
                    in_=osb[:, 0:half_out, :])
        nc.sync.dma_start(
            out=out_d[b, half_out * 128:].rearrange("(ic p) d -> p ic d", p=128),
            in_=osb[:, half_out:, :])


def prep_inputs(x, adj, W, a_src, a_dst):
    """Host-side prep: shard x over cores, build combined weight layouts."""
    x = np.asarray(x, np.float32)
    adj = np.asarray(adj)
    W = np.asarray(W, np.float32)
    a_src = np.asarray(a_src, np.float32)
    a_dst = np.asarray(a_dst, np.float32)

    maskT = np.ascontiguousarray(adj.T).astype(NPBF)
    Acat = np.zeros((HD, 36), np.float32)
    for h in range(H):
        Acat[h * D:(h + 1) * D, h] = a_src[h]
        Acat[h * D:(h + 1) * D, 32 + h] = a_dst[h]
    WAcat = np.ascontiguousarray(W @ Acat)  # (IN_DIM, 36): src at 0-3, dst at 32-35

    onehot = np.zeros((4, 4 * 128), np.float32)
    for h in range(H):
        onehot[h, h * 128:(h + 1) * 128] = 1.0

    in_maps = []
    for c in range(NCORES):
        xT = np.ascontiguousarray(x[c * BL:(c + 1) * BL].transpose(0, 2, 1))
        in_maps.append({"xT": xT, "W": W, "WAcat": WAcat, "maskT": maskT,
                        "onehot": onehot})
    return in_maps


_PROGRAM_CACHE = {}


def _get_program():
    if "nc" not in _PROGRAM_CACHE:
        _PROGRAM_CACHE["nc"] = build_gat_program()
    return _PROGRAM_CACHE["nc"]


def run_on_hw(inputs, trace=False):
    from concourse.bass_utils import run_bass_kernel_spmd
    nc = _get_program()
    in_maps = prep_inputs(**inputs)
    res = run_bass_kernel_spmd(nc, in_maps, list(range(NCORES)), trace=trace)
    out = np.concatenate(
        [np.asarray(res.results[c]["out"]).astype(np.float32)
         for c in range(NCORES)], axis=0)
    return out, res


def kernel(**inputs) -> np.ndarray:
    out, _ = run_on_hw(inputs, trace=False)
    return out
